# revision 1
# baseline (speedup 1.0000x reference)
"""Trainium2 Bass kernel for BaseBertSelfAttention (B=2, S=2048, H=1024, 16 heads).

Sharding (8 NeuronCores):
  - Tensor-parallel on heads: core c owns heads (2c, 2c+1) -> d_local = 128.
  - Each core: QKV projections (column-parallel) for its 2 heads over BOTH
    batches, attention in transposed layout (scores^T: keys on partitions,
    queries on the free axis), softmax denominator via a ones-augmented V
    column, normalized context ctx^T [d_local=128, B*S].
  - One 8-rank AllToAll redistributes ctx^T from head-sharding to
    row-sharding (1 MB/core, bf16).
  - Each core then computes Wo projection + residual + LayerNorm for its 512
    output rows (row j of 8 chunks of the flattened [B*S, H] output).

Precision: bf16 matmul inputs (4x PE throughput), fp32 PSUM accumulation,
fp32 softmax denominators / reciprocal / residual / LayerNorm.  The final
output is dominated by the fp32 residual + LayerNorm path, so end-to-end
relative error stays ~1e-4.
"""

import numpy as np
import ml_dtypes

import concourse.bass as bass
import concourse.tile as tile
from concourse import bacc, mybir
from concourse.bass_utils import run_bass_kernel_spmd

BF16 = mybir.dt.bfloat16
FP8 = mybir.dt.float8e4
F32 = mybir.dt.float32
AF = mybir.ActivationFunctionType
P = 128

B, S, H = 2, 2048, 1024
NH, HD = 16, 64
NCORES = 8
EPS = 1e-12
SCALE = 1.0 / 8.0  # 1/sqrt(HD)

_CACHE: dict = {}


def _build_program(s=S):
    """Build the (identical-across-cores) Bass program."""
    nkb = s // P               # key blocks of 128
    qc_per_b = NCORES // B     # q chunks per batch (4)
    rows = (B * s) // NCORES   # output rows per core / q-chunk width (512)
    qw = rows
    ho = H // P                # h chunks of 128 (8)

    nc = bacc.Bacc("TRN2", target_bir_lowering=False, debug=False,
                   num_devices=NCORES)
    xT = nc.dram_tensor("xT", [B, H, s], BF16, kind="ExternalInput")
    wq = nc.dram_tensor("wq", [H, P], BF16, kind="ExternalInput")
    wk = nc.dram_tensor("wk", [H, P], BF16, kind="ExternalInput")
    wv = nc.dram_tensor("wv", [H, P], BF16, kind="ExternalInput")
    wo = nc.dram_tensor("wo", [H, H], FP8, kind="ExternalInput")
    bq = nc.dram_tensor("bq", [P], F32, kind="ExternalInput")
    bk = nc.dram_tensor("bk", [P], F32, kind="ExternalInput")
    bv = nc.dram_tensor("bv", [P, P], F32, kind="ExternalInput")
    maskT = nc.dram_tensor("maskT", [B, P, nkb], F32, kind="ExternalInput")
    xres = nc.dram_tensor("xres", [rows, H], F32, kind="ExternalInput")
    gamma = nc.dram_tensor("gamma", [P, H], F32, kind="ExternalInput")
    beta = nc.dram_tensor("beta", [P, H], F32, kind="ExternalInput")
    out = nc.dram_tensor("out", [rows, H], F32, kind="ExternalOutput")

    with tile.TileContext(nc) as tc:
        _kernel_body(
            tc, s, nkb, qw, qc_per_b, rows, ho,
            xT, wq, wk, wv, wo, bq, bk, bv, maskT, xres, gamma, beta, out,
        )
    nc.compile()
    return nc


def _kernel_body(tc, s, nkb, qw, qc_per_b, rows, ho,
                 xT, wq, wk, wv, wo, bq, bk, bv, maskT, xres, gamma, beta, out):
    nc = tc.nc
    VPAD = 80  # padded free width of the ones-augmented V tiles (65 used)

    import contextlib
    stack = contextlib.ExitStack()
    with stack:
        consts = stack.enter_context(tc.tile_pool(name="consts", bufs=1))
        dram = stack.enter_context(tc.tile_pool(name="dram", bufs=1, space="DRAM"))

        # ---------------- constant / input loads ----------------
        wq_sb = consts.tile([P, ho, P], BF16)
        wk_sb = consts.tile([P, ho, P], BF16)
        wv_sb = consts.tile([P, ho, P], BF16)
        nc.sync.dma_start(wq_sb, wq.rearrange("(o p) d -> p o d", p=P))
        nc.sync.dma_start(wk_sb, wk.rearrange("(o p) d -> p o d", p=P))
        nc.sync.dma_start(wv_sb, wv.rearrange("(o p) d -> p o d", p=P))
        wo_sb = consts.tile([P, ho, H], FP8)

        bq_sb = consts.tile([P, 1], F32)
        bk_sb = consts.tile([P, 1], F32)
        nc.sync.dma_start(bq_sb, bq[:, None])
        nc.sync.dma_start(bk_sb, bk[:, None])
        # host-prepared partition-broadcast tiles
        bv_b = consts.tile([P, P], F32)
        nc.sync.dma_start(bv_b, bv[:, :])
        gamma_b = consts.tile([P, H], F32)
        nc.sync.dma_start(gamma_b, gamma[:, :])
        beta_b = consts.tile([P, H], F32)
        nc.sync.dma_start(beta_b, beta[:, :])

        mask_sb = consts.tile([P, B, nkb], F32)
        nc.sync.dma_start(mask_sb, maskT.rearrange("b p k -> p b k"))

        ones_sb = consts.tile([P, P], BF16)
        nc.vector.memset(ones_sb, 1.0)
        eps_sb = consts.tile([P, 1], F32)
        nc.vector.memset(eps_sb, EPS)

        # x^T (bf16): [p(h-inner), b, h-outer, s]; load per (b, o) for pipelining
        xT_sb = consts.tile([P, B, ho, s], BF16)
        xT_r = xT.rearrange("b (o p) s -> p b o s", p=P)
        for b in range(B):
            for o in range(ho):
                nc.sync.dma_start(xT_sb[:, b, o, :], xT_r[:, b, o, :])

        xres_sb = consts.tile([P, rows // P, H], F32)

        # attention intermediates
        qT_sb = consts.tile([P, B, s], BF16)   # Q^T [d_local, b, s]
        kT_sb = consts.tile([P, B, s], BF16)   # K^T [d_local, b, s]
        # ones-augmented V (natural layout), per head: [p(s-inner), b, kb, 65]
        v_e = consts.tile([P, B, nkb, VPAD], BF16)
        v_o = consts.tile([P, B, nkb, VPAD], BF16)
        nc.vector.memset(v_e, 1.0)
        nc.vector.memset(v_o, 1.0)
        # denominator column carries 1/256 so ctx*recip lands in fp8 range
        nc.vector.memset(v_e[:, :, :, 64:65], 1.0 / 256.0)
        nc.vector.memset(v_o[:, :, :, 64:65], 1.0 / 256.0)

        # A2A bounce buffers (DRAM, local)
        a2a_in = dram.tile([NCORES * P, qw], FP8)
        a2a_out = dram.tile([NCORES * P, qw], FP8)

        # PSUM: qk pool 1 bank (QKV+Wo), s pool 4 banks, ctx pool 3 banks (+E)
        ps_qk = stack.enter_context(tc.tile_pool(name="ps_qk", bufs=1, space="PSUM"))
        ps_s = stack.enter_context(tc.tile_pool(name="ps_s", bufs=2, space="PSUM"))
        ps_ctx = stack.enter_context(tc.tile_pool(name="ps_ctx", bufs=3, space="PSUM"))
        ptile = stack.enter_context(tc.tile_pool(name="ptile", bufs=6))
        misc = stack.enter_context(tc.tile_pool(name="misc", bufs=3))

        def qkv_stage(b):
            # interleave Q/K chunk projections with the V blocks they cover so
            # attention's AV matmuls are never starved waiting for V
            kb_per_sc = nkb // (s // 512)
            for sc in range(s // 512):
                sl = slice(sc * 512, (sc + 1) * 512)
                for w_sb, bias_sb, dst in (
                    (wq_sb, bq_sb, qT_sb),
                    (wk_sb, bk_sb, kT_sb),
                ):
                    ps = ps_qk.tile([P, 512], F32, tag="qk")
                    for o in range(ho):
                        nc.tensor.matmul(
                            ps, lhsT=w_sb[:, o, :], rhs=xT_sb[:, b, o, sl],
                            start=(o == 0), stop=(o == ho - 1))
                    nc.vector.tensor_tensor(
                        dst[:, b, sl], ps,
                        bias_sb[:, 0:1].to_broadcast((P, 512)),
                        mybir.AluOpType.add)
                for kb in range(sc * kb_per_sc, (sc + 1) * kb_per_sc):
                    ksl = slice(kb * P, (kb + 1) * P)
                    ps = ps_qk.tile([P, 512], F32, tag="qk")
                    for o in range(ho):
                        nc.tensor.matmul(
                            ps[:, 0:P], lhsT=xT_sb[:, b, o, ksl],
                            rhs=wv_sb[:, o, :],
                            start=(o == 0), stop=(o == ho - 1))
                    nc.vector.tensor_tensor(
                        v_e[:, b, kb, 0:64], ps[:, 0:64], bv_b[:, 0:64],
                        mybir.AluOpType.add)
                    nc.vector.tensor_tensor(
                        v_o[:, b, kb, 0:64], ps[:, 64:128], bv_b[:, 64:128],
                        mybir.AluOpType.add)

        def attn_stage(b):
            if True:
                for qc in range(qc_per_b):
                    qsl = slice(qc * qw, (qc + 1) * qw)
                    shard = b * qc_per_b + qc
                    ctx_e = ps_ctx.tile([P, qw], F32, tag="ctx")
                    ctx_o = ps_ctx.tile([P, qw], F32, tag="ctx")
                    for kb in range(nkb):
                        ksl = slice(kb * P, (kb + 1) * P)
                        sp = ps_s.tile([P, 2, qw], F32, tag="s")
                        nc.tensor.matmul(
                            sp[:, 0, :], lhsT=kT_sb[0:64, b, ksl],
                            rhs=qT_sb[0:64, b, qsl], start=True, stop=True)
                        nc.tensor.matmul(
                            sp[:, 1, :], lhsT=kT_sb[64:128, b, ksl],
                            rhs=qT_sb[64:128, b, qsl], start=True, stop=True)
                        pp = ptile.tile([P, 2, qw], BF16, tag="p")
                        nc.scalar.activation(
                            pp, sp, AF.Exp,
                            bias=mask_sb[:, b, kb:kb + 1], scale=SCALE)
                        nc.tensor.matmul(
                            ctx_e[0:65, :], lhsT=v_e[:, b, kb, 0:65],
                            rhs=pp[:, 0, :], start=(kb == 0), stop=(kb == nkb - 1),
                            skip_group_check=True)
                        nc.tensor.matmul(
                            ctx_o[0:65, :], lhsT=v_o[:, b, kb, 0:65],
                            rhs=pp[:, 1, :], start=(kb == 0), stop=(kb == nkb - 1),
                            skip_group_check=True)
                    # normalize: ctx[d, q] / denom[q]  (denom = row 64)
                    for h, ctx_ps in enumerate((ctx_e, ctx_o)):
                        rinv = misc.tile([1, qw], F32, tag="rinv")
                        nc.vector.reciprocal(rinv, ctx_ps[64:65, :])
                        rb = misc.tile([1, qw], BF16, tag="rb")
                        nc.vector.tensor_copy(out=rb, in_=rinv)
                        eb = ps_ctx.tile([64, qw], F32, tag="ctx")
                        nc.tensor.matmul(
                            eb, lhsT=ones_sb[0:1, 0:64], rhs=rb,
                            start=True, stop=True)
                        ctx_bf = misc.tile([64, qw], BF16, tag="cb")
                        nc.vector.tensor_copy(out=ctx_bf, in_=ctx_ps[0:64, :])
                        cn = misc.tile([64, qw], FP8, tag="cn")
                        nc.vector.tensor_tensor(
                            cn, ctx_bf, eb, mybir.AluOpType.mult)
                        r0 = shard * P + h * 64
                        nc.sync.dma_start(a2a_in[r0:r0 + 64, :], cn)

        for b in range(B):
            qkv_stage(b)
            attn_stage(b)

        # tail-stage inputs (emitted late => low DMA priority, still prefetched)
        nc.sync.dma_start(wo_sb, wo.rearrange("(o p) n -> p o n", p=P))
        nc.sync.dma_start(xres_sb, xres.rearrange("(r p) h -> p r h", p=P))

        # ---------------- stage 3: AllToAll ----------------
        nc.gpsimd.collective_compute(
            "AllToAll", mybir.AluOpType.bypass,
            replica_groups=[list(range(NCORES))],
            ins=[a2a_in[:].opt()], outs=[a2a_out[:].opt()])

        # ---------------- stage 4: Wo + residual + LayerNorm ----------------
        ctxf = consts.tile([P, ho, qw], FP8)
        for o in range(ho):
            nc.sync.dma_start(
                ctxf[:, o, :],
                a2a_out.rearrange("(o p) q -> p o q", p=P)[:, o, :])
        with tc.tile_pool(name="fin", bufs=2) as fin:
            for qt in range(rows // P):
                res = fin.tile([P, H], F32, tag="res")
                for nch in range(H // 512):
                    nsl = slice(nch * 512, (nch + 1) * 512)
                    ps = ps_ctx.tile([P, 512], F32, tag="ctx")
                    for o in range(ho):
                        nc.tensor.matmul(
                            ps, lhsT=ctxf[:, o, qt * P:(qt + 1) * P],
                            rhs=wo_sb[:, o, nsl],
                            start=(o == 0), stop=(o == ho - 1))
                    nc.vector.tensor_tensor(
                        res[:, nsl], ps, xres_sb[:, qt, nsl],
                        mybir.AluOpType.add)
                # LayerNorm over H (free axis)
                stats = fin.tile([P, H // 512, 6], F32, tag="st")
                for g in range(H // 512):
                    nc.vector.bn_stats(
                        stats[:, g, :], res[:, g * 512:(g + 1) * 512])
                mv = fin.tile([P, 2], F32, tag="mv")
                nc.vector.bn_aggr(out=mv, in_=stats)
                rstd = fin.tile([P, 1], F32, tag="rstd")
                nc.scalar.activation(rstd, mv[:, 1:2], AF.Sqrt, bias=eps_sb)
                nc.vector.reciprocal(rstd, rstd)
                nc.vector.tensor_tensor(
                    res, res, mv[:, 0:1].to_broadcast((P, H)),
                    mybir.AluOpType.subtract)
                nc.vector.tensor_tensor(
                    res, res, rstd[:, 0:1].to_broadcast((P, H)),
                    mybir.AluOpType.mult)
                outt = fin.tile([P, H], F32, tag="outt")
                nc.gpsimd.tensor_tensor(outt, res, gamma_b, mybir.AluOpType.mult)
                nc.gpsimd.tensor_tensor(outt, outt, beta_b, mybir.AluOpType.add)
                nc.sync.dma_start(out[qt * P:(qt + 1) * P, :], outt)


def get_program(s=S):
    key = ("nc", s)
    if key not in _CACHE:
        _CACHE[key] = _build_program(s)
    return _CACHE[key]


def make_in_maps(hidden_states, attention_mask, Wq, bq, Wk, bk, Wv, bv, Wo, bo,
                 ln_gamma, ln_beta):
    """Host-side sharding: build the 8 per-core input maps."""
    bf = ml_dtypes.bfloat16
    hs = np.asarray(hidden_states, dtype=np.float32)
    b_, s_, h_ = hs.shape
    nkb = s_ // P
    rows = (b_ * s_) // NCORES
    qc_per_b = NCORES // b_

    xT = np.ascontiguousarray(hs.transpose(0, 2, 1)).astype(bf)  # [B, H, S]
    Wq = np.asarray(Wq, np.float32)
    Wk = np.asarray(Wk, np.float32)
    Wv = np.asarray(Wv, np.float32)
    wo_f8 = np.ascontiguousarray(
        np.asarray(Wo, np.float32) * 256.0).astype(ml_dtypes.float8_e4m3)
    bq = np.asarray(bq, np.float32)
    bk = np.asarray(bk, np.float32)
    bv = np.asarray(bv, np.float32)
    bo = np.asarray(bo, np.float32)
    gamma_bc = np.ascontiguousarray(
        np.broadcast_to(np.asarray(ln_gamma, np.float32)[None, :], (P, H)))
    beta_bc = np.ascontiguousarray(
        np.broadcast_to(np.asarray(ln_beta, np.float32)[None, :], (P, H)))
    mask = np.asarray(attention_mask, np.float32).reshape(b_, s_)
    maskT = np.ascontiguousarray(
        mask.reshape(b_, nkb, P).transpose(0, 2, 1))  # [B, P, nkb]

    in_maps = []
    for c in range(NCORES):
        d0 = c * P
        b_out, j = divmod(c, qc_per_b)
        rsl = slice(j * rows, (j + 1) * rows)
        in_maps.append({
            "xT": xT,
            "wq": np.ascontiguousarray(Wq[:, d0:d0 + P]).astype(bf),
            "wk": np.ascontiguousarray(Wk[:, d0:d0 + P]).astype(bf),
            "wv": np.ascontiguousarray(Wv[:, d0:d0 + P]).astype(bf),
            "wo": wo_f8,
            "bq": np.ascontiguousarray(bq[d0:d0 + P]),
            "bk": np.ascontiguousarray(bk[d0:d0 + P]),
            "bv": np.ascontiguousarray(
                np.broadcast_to(bv[d0:d0 + P][None, :], (P, P))),
            "maskT": maskT,
            "xres": np.ascontiguousarray(
                (hs[b_out, rsl, :] + bo[None, :]) * 65536.0),
            "gamma": gamma_bc,
            "beta": beta_bc,
        })
    return in_maps


def assemble_output(results, b_=B, s_=S, h_=H):
    rows = (b_ * s_) // NCORES
    qc_per_b = NCORES // b_
    out = np.empty((b_, s_, h_), np.float32)
    for c in range(NCORES):
        b_out, j = divmod(c, qc_per_b)
        out[b_out, j * rows:(j + 1) * rows, :] = np.asarray(
            results[c]["out"], np.float32)
    return out


def kernel(**inputs):
    nc = get_program(S)
    in_maps = make_in_maps(**inputs)
    res = run_bass_kernel_spmd(nc, in_maps, list(range(NCORES)))
    return assemble_output(res.results)



# revision 8
# speedup vs baseline: 415.8165x; 415.8165x over previous
"""Trainium2 Bass kernel for BaseBertSelfAttention (B=2, S=2048, H=1024, 16 heads).

Sharding (8 NeuronCores):
  - Tensor-parallel on heads: core c owns heads (2c, 2c+1) -> d_local = 128.
  - Each core: QKV projections (column-parallel) for its 2 heads over BOTH
    batches, attention in transposed layout (scores^T: keys on partitions,
    queries on the free axis), softmax denominator via a ones-augmented V
    column, normalized context ctx^T [d_local=128, B*S].
  - Output rows are interleaved by batch: core c owns rows
    [c*256,(c+1)*256) of EACH batch.  This lets one 8-rank AllToAll per
    batch redistribute ctx^T from head-sharding to row-sharding; the
    batch-0 AllToAll and its Wo+LayerNorm tail fully overlap with the
    batch-1 QKV/attention compute, leaving only the (half-size) batch-1
    collective + tail exposed.
  - Each core then computes Wo projection + residual + LayerNorm for its
    2x128 output rows per batch.

Precision: bf16 matmul inputs (4x PE throughput), fp32 PSUM accumulation,
fp32 softmax denominators / reciprocal / residual / LayerNorm.  ctx and Wo
travel as scaled fp8 (the LayerNorm normalization cancels the scale).
"""

import numpy as np
import ml_dtypes

import concourse.bass as bass
import concourse.tile as tile
from concourse import bacc, mybir
from concourse.bass_utils import run_bass_kernel_spmd

BF16 = mybir.dt.bfloat16
FP8 = mybir.dt.float8e4
F32 = mybir.dt.float32
AF = mybir.ActivationFunctionType
ALU = mybir.AluOpType
P = 128

B, S, H = 2, 2048, 1024
NH, HD = 16, 64
NCORES = 8
EPS = 1e-12
SCALE = 1.0 / 8.0  # 1/sqrt(HD)
RPB = (B * S) // (NCORES * B)  # rows per (core, batch) = 256

_CACHE: dict = {}


def _build_program(s=S, repeat=1):
    """Build the (identical-across-cores) Bass program.

    repeat>1 replays the whole compute body that many times (same inputs,
    same output) -- used only by the timing harness to measure per-iteration
    device time with dispatch overhead amortized away.
    """
    nkb = s // P               # key blocks of 128
    qc_per_b = 4               # q chunks per batch (512 wide each)
    qw = s // qc_per_b         # 512
    rows = (B * s) // NCORES   # output rows per core (256 per batch)
    rpb = rows // B            # 256
    ho = H // P                # h chunks of 128 (8)

    nc = bacc.Bacc("TRN2", target_bir_lowering=False, debug=False,
                   num_devices=NCORES)
    xT = nc.dram_tensor("xT", [B, H, s], BF16, kind="ExternalInput")
    wq = nc.dram_tensor("wq", [H, P], BF16, kind="ExternalInput")
    wk = nc.dram_tensor("wk", [H, P], BF16, kind="ExternalInput")
    wv = nc.dram_tensor("wv", [H, P], BF16, kind="ExternalInput")
    wo = nc.dram_tensor("wo", [H, H], FP8, kind="ExternalInput")
    bq = nc.dram_tensor("bq", [P], F32, kind="ExternalInput")
    bk = nc.dram_tensor("bk", [P], F32, kind="ExternalInput")
    bv = nc.dram_tensor("bv", [P, P], F32, kind="ExternalInput")
    maskT = nc.dram_tensor("maskT", [B, P, nkb], F32, kind="ExternalInput")
    xres = nc.dram_tensor("xres", [rows, H], F32, kind="ExternalInput")
    gamma = nc.dram_tensor("gamma", [P, H], F32, kind="ExternalInput")
    beta = nc.dram_tensor("beta", [P, H], F32, kind="ExternalInput")
    out = nc.dram_tensor("out", [rows, H], F32, kind="ExternalOutput")

    with tile.TileContext(nc) as tc:
        _kernel_body(
            tc, s, nkb, qw, qc_per_b, rows, rpb, ho, repeat,
            xT, wq, wk, wv, wo, bq, bk, bv, maskT, xres, gamma, beta, out,
        )
    nc.compile()
    return nc


def _kernel_body(tc, s, nkb, qw, qc_per_b, rows, rpb, ho, repeat,
                 xT, wq, wk, wv, wo, bq, bk, bv, maskT, xres, gamma, beta, out):
    nc = tc.nc
    VPAD = 80  # padded free width of the ones-augmented V tiles (65 used)

    import contextlib
    stack = contextlib.ExitStack()
    with stack:
        consts = stack.enter_context(tc.tile_pool(name="consts", bufs=1))
        dram = stack.enter_context(tc.tile_pool(name="dram", bufs=2, space="DRAM"))

        # ---------------- constant / input loads ----------------
        wq_sb = consts.tile([P, ho, P], BF16)
        wk_sb = consts.tile([P, ho, P], BF16)
        wv_sb = consts.tile([P, ho, P], BF16)
        xT_sb = consts.tile([P, B, ho, s], BF16)
        xT_r = xT.rearrange("b (o p) s -> p b o s", p=P)

        # startup-critical order: wq, first x chunk, wk/wv, rest of x
        nc.sync.dma_start(wq_sb, wq.rearrange("(o p) d -> p o d", p=P))
        for o in range(ho):
            nc.sync.dma_start(xT_sb[:, 0, o, 0:512], xT_r[:, 0, o, 0:512])
        nc.sync.dma_start(wk_sb, wk.rearrange("(o p) d -> p o d", p=P))
        nc.sync.dma_start(wv_sb, wv.rearrange("(o p) d -> p o d", p=P))

        bq_sb = consts.tile([P, 1], F32)
        bk_sb = consts.tile([P, 1], F32)
        nc.sync.dma_start(bq_sb, bq[:, None])
        nc.sync.dma_start(bk_sb, bk[:, None])
        # host-prepared partition-broadcast tiles
        bv_b = consts.tile([P, P], F32)
        nc.sync.dma_start(bv_b, bv[:, :])
        mask_sb = consts.tile([P, B, nkb], F32)
        nc.sync.dma_start(mask_sb, maskT.rearrange("b p k -> p b k"))

        ones_sb = consts.tile([P, P], BF16)
        nc.vector.memset(ones_sb, 1.0)
        eps_sb = consts.tile([P, 1], F32)
        nc.vector.memset(eps_sb, EPS)

        # x^T (bf16): [p(h-inner), b, h-outer, s]; fine-grained loads in
        # consumption order (b, sc, o) so the first QKV chunk starts early
        for b in range(B):
            for sc in range(s // 512):
                if b == 0 and sc == 0:
                    continue
                sl = slice(sc * 512, (sc + 1) * 512)
                for o in range(ho):
                    nc.sync.dma_start(xT_sb[:, b, o, sl], xT_r[:, b, o, sl])

        wo_sb = consts.tile([P, ho, H], FP8)
        xres_sb = consts.tile([P, B, rpb // P, H], F32)
        gamma_b = consts.tile([P, H], F32)
        beta_b = consts.tile([P, H], F32)

        # attention intermediates
        qT_sb = consts.tile([P, B, s], BF16)   # Q^T [d_local, b, s]
        kT_sb = consts.tile([P, B, s], BF16)   # K^T [d_local, b, s]
        # ones-augmented V (natural layout), per head: [p(s-inner), b, kb, 65]
        v_e = consts.tile([P, B, nkb, VPAD], BF16)
        v_o = consts.tile([P, B, nkb, VPAD], BF16)
        nc.vector.memset(v_e, 1.0)
        nc.vector.memset(v_o, 1.0)
        # denominator column carries 1/256 so ctx*recip lands in fp8 range
        nc.vector.memset(v_e[:, :, :, 64:65], 1.0 / 256.0)
        nc.vector.memset(v_o[:, :, :, 64:65], 1.0 / 256.0)

        ctxf = consts.tile([P, B, ho, rpb], FP8)

        # PSUM: qk 2 banks (QKV drains + Wo + eb), s 2x2 banks, ctx 2 banks
        ps_qk = stack.enter_context(tc.tile_pool(name="ps_qk", bufs=2, space="PSUM"))
        ps_s = stack.enter_context(tc.tile_pool(name="ps_s", bufs=2, space="PSUM"))
        ps_ctx = stack.enter_context(tc.tile_pool(name="ps_ctx", bufs=2, space="PSUM"))
        ptile = stack.enter_context(tc.tile_pool(name="ptile", bufs=4))
        misc = stack.enter_context(tc.tile_pool(name="misc", bufs=3))
        fin = stack.enter_context(tc.tile_pool(name="fin", bufs=2))

        def qkv_proj(b, sc, w_sb, bias_sb, dst):
            sl = slice(sc * 512, (sc + 1) * 512)
            ps = ps_qk.tile([P, 512], F32, tag="qk")
            for o in range(ho):
                nc.tensor.matmul(
                    ps, lhsT=w_sb[:, o, :], rhs=xT_sb[:, b, o, sl],
                    start=(o == 0), stop=(o == ho - 1))
            nc.vector.tensor_tensor(
                dst[:, b, sl], ps,
                bias_sb[:, 0:1].to_broadcast((P, 512)),
                ALU.add)

        def qkv_vblock(b, kb):
            ksl = slice(kb * P, (kb + 1) * P)
            ps = ps_qk.tile([P, 512], F32, tag="qk")
            for o in range(ho):
                nc.tensor.matmul(
                    ps[:, 0:P], lhsT=xT_sb[:, b, o, ksl],
                    rhs=wv_sb[:, o, :],
                    start=(o == 0), stop=(o == ho - 1))
            nc.vector.tensor_tensor(
                v_e[:, b, kb, 0:64], ps[:, 0:64], bv_b[:, 0:64],
                ALU.add)
            nc.vector.tensor_tensor(
                v_o[:, b, kb, 0:64], ps[:, 64:128], bv_b[:, 64:128],
                ALU.add)

        def qkv_chunks(b):
            """QKV work split into ~1.7us PE chunks for interleaved emission."""
            kb_per_sc = nkb // (s // 512)
            for sc in range(s // 512):
                yield lambda sc=sc: qkv_proj(b, sc, wq_sb, bq_sb, qT_sb)
                yield lambda sc=sc: qkv_proj(b, sc, wk_sb, bk_sb, kT_sb)
                for kb in range(sc * kb_per_sc, (sc + 1) * kb_per_sc):
                    yield lambda kb=kb: qkv_vblock(b, kb)

        def qkv_stage(b):
            for ch in qkv_chunks(b):
                ch()

        def attn_qc(b, qc, a2a_in, filler=None):
            qsl = slice(qc * qw, (qc + 1) * qw)
            ctx_e = ps_ctx.tile([P, qw], F32, tag="ctx")
            ctx_o = ps_ctx.tile([P, qw], F32, tag="ctx")
            for kb in range(nkb):
                if filler is not None:
                    filler(qc * nkb + kb)
                ksl = slice(kb * P, (kb + 1) * P)
                sp = ps_s.tile([P, 2, qw], F32, tag="s")
                nc.tensor.matmul(
                    sp[:, 0, :], lhsT=kT_sb[0:64, b, ksl],
                    rhs=qT_sb[0:64, b, qsl], start=True, stop=True)
                nc.tensor.matmul(
                    sp[:, 1, :], lhsT=kT_sb[64:128, b, ksl],
                    rhs=qT_sb[64:128, b, qsl], start=True, stop=True)
                pp = ptile.tile([P, 2, qw], BF16, tag="p")
                nc.scalar.activation(
                    pp, sp, AF.Exp,
                    bias=mask_sb[:, b, kb:kb + 1], scale=SCALE)
                nc.tensor.matmul(
                    ctx_e[0:65, :], lhsT=v_e[:, b, kb, 0:65],
                    rhs=pp[:, 0, :], start=(kb == 0), stop=(kb == nkb - 1),
                    skip_group_check=True)
                nc.tensor.matmul(
                    ctx_o[0:65, :], lhsT=v_o[:, b, kb, 0:65],
                    rhs=pp[:, 1, :], start=(kb == 0), stop=(kb == nkb - 1),
                    skip_group_check=True)
            # normalize: ctx[d, q] / denom[q]  (denom = row 64), emit to
            # the two destination shards this q-chunk covers
            for h, ctx_ps in enumerate((ctx_e, ctx_o)):
                rinv = misc.tile([1, qw], F32, tag="rinv")
                nc.vector.reciprocal(rinv, ctx_ps[64:65, :])
                rb = misc.tile([1, qw], BF16, tag="rb")
                nc.vector.tensor_copy(out=rb, in_=rinv)
                eb = ps_qk.tile([64, qw], F32, tag="qk")
                nc.tensor.matmul(
                    eb, lhsT=ones_sb[0:1, 0:64], rhs=rb,
                    start=True, stop=True)
                ctx_bf = misc.tile([64, qw], BF16, tag="cb")
                nc.vector.tensor_copy(out=ctx_bf, in_=ctx_ps[0:64, :])
                cn = misc.tile([64, qw], FP8, tag="cn")
                nc.vector.tensor_tensor(cn, ctx_bf, eb, ALU.mult)
                for half in range(qw // rpb):
                    dest = qc * (qw // rpb) + half
                    r0 = dest * P + h * 64
                    nc.sync.dma_start(
                        a2a_in[r0:r0 + 64, :],
                        cn[:, half * rpb:(half + 1) * rpb])

        def fin_wo(b, qt):
            """Wo projection + residual for one 128-row tile -> res tile."""
            res = fin.tile([P, H], F32, tag="res")
            for nch in range(H // 512):
                nsl = slice(nch * 512, (nch + 1) * 512)
                ps = ps_qk.tile([P, 512], F32, tag="qk")
                for o in range(ho):
                    nc.tensor.matmul(
                        ps, lhsT=ctxf[:, b, o, qt * P:(qt + 1) * P],
                        rhs=wo_sb[:, o, nsl],
                        start=(o == 0), stop=(o == ho - 1))
                nc.vector.tensor_tensor(
                    res[:, nsl], ps, xres_sb[:, b, qt, nsl],
                    ALU.add)
            return res

        def fin_ln(b, qt, res):
            """LayerNorm over H (free axis) + store. DVE/Pool only -- the
            rstd is a Newton rsqrt so the ACT Exp table is never evicted."""
            stats = fin.tile([P, H // 512, 6], F32, tag="st")
            for g in range(H // 512):
                nc.vector.bn_stats(
                    stats[:, g, :], res[:, g * 512:(g + 1) * 512])
            mv = fin.tile([P, 2], F32, tag="mv")
            nc.vector.bn_aggr(out=mv, in_=stats)
            # y = rsqrt(var + eps) via Newton iterations (seed 1/65536:
            # rows are pre-scaled by 65536 and have ~unit variance)
            x = fin.tile([P, 1], F32, tag="x")
            nc.vector.tensor_tensor(x, mv[:, 1:2], eps_sb, ALU.add)
            y = fin.tile([P, 1], F32, tag="y")
            nc.vector.memset(y, 1.0 / 65536.0)
            t = fin.tile([P, 1], F32, tag="t")
            for _ in range(4):
                nc.vector.tensor_tensor(t, x, y, ALU.mult)
                nc.vector.tensor_tensor(t, t, y, ALU.mult)
                nc.vector.tensor_scalar(t, t, -0.5, 1.5, ALU.mult, ALU.add)
                nc.vector.tensor_tensor(y, y, t, ALU.mult)
            # (res - mu) * rstd in a single DVE pass
            nc.vector.tensor_scalar(
                res, res, mv[:, 0:1], y[:, 0:1],
                ALU.subtract, ALU.mult)
            outt = fin.tile([P, H], F32, tag="outt")
            nc.vector.tensor_tensor(outt, res, gamma_b, ALU.mult)
            nc.gpsimd.tensor_tensor(outt, outt, beta_b, ALU.add)
            nc.sync.dma_start(
                out[(b * (rpb // P) + qt) * P:(b * (rpb // P) + qt + 1) * P, :],
                outt)

        def do_a2a(pair):
            nc.gpsimd.collective_compute(
                "AllToAll", ALU.bypass,
                replica_groups=[list(range(NCORES))],
                ins=[pair[0][:].opt()], outs=[pair[1][:].opt()])

        def ctxf_load(b, a2a_out):
            for o in range(ho):
                nc.sync.dma_start(
                    ctxf[:, b, o, :],
                    a2a_out.rearrange("(o p) q -> p o q", p=P)[:, o, :])

        def make_filler(chunks, total_slots, start=0):
            """Spread chunk emission across attention kb slots >= start."""
            chunks = list(chunks)
            state = {"done": 0}
            n = len(chunks)

            def fill(g):
                if g < start:
                    return
                want = min(n, ((g - start + 1) * n) // max(1, total_slots - start))
                while state["done"] < want:
                    chunks[state["done"]]()
                    state["done"] += 1

            def flush():
                while state["done"] < len(chunks):
                    chunks[state["done"]]()
                    state["done"] += 1

            return fill, flush

        # preload the Exp activation table during the startup DMA wait
        warm = misc.tile([1, 1], F32, tag="warm")
        nc.scalar.activation(warm, eps_sb[0:1, :], AF.Exp)

        nslots = qc_per_b * nkb
        for it in range(repeat):
            a2a = []
            for b in range(B):
                a2a_i = dram.tile([NCORES * P, rpb], FP8, tag="a2ai",
                                  name=f"a2a_in_{it}_{b}")
                a2a_o = dram.tile([NCORES * P, rpb], FP8, tag="a2ao",
                                  name=f"a2a_out_{it}_{b}")
                a2a.append((a2a_i, a2a_o))
            # batch 0 QKV, then batch-0 attention with batch-1 QKV chunks
            # interleaved into the exp-wait bubbles of the kb loop
            qkv_stage(0)
            fill1, flush1 = make_filler(qkv_chunks(1), nslots)
            for qc in range(qc_per_b):
                attn_qc(0, qc, a2a[0][0], filler=fill1)
            flush1()
            do_a2a(a2a[0])
            ctxf_load(0, a2a[0][1])
            if it == 0:
                # tail-stage inputs (late emission => low DMA priority)
                nc.sync.dma_start(wo_sb, wo.rearrange("(o p) n -> p o n", p=P))
                nc.sync.dma_start(
                    xres_sb,
                    xres.rearrange("(b r p) h -> p b r h", p=P, b=B))
                nc.sync.dma_start(gamma_b, gamma[:, :])
                nc.sync.dma_start(beta_b, beta[:, :])
            # batch-1 attention with the batch-0 tail (Wo + LayerNorm)
            # interleaved into its second half (after the b0 collective lands)
            resh = {}
            fin0 = [
                lambda: resh.__setitem__(0, fin_wo(0, 0)),
                lambda: fin_ln(0, 0, resh[0]),
                lambda: resh.__setitem__(1, fin_wo(0, 1)),
                lambda: fin_ln(0, 1, resh[1]),
            ]
            fill2, flush2 = make_filler(fin0, nslots, start=int(nslots * 0.45))
            for qc in range(qc_per_b):
                attn_qc(1, qc, a2a[1][0], filler=fill2)
            flush2()
            do_a2a(a2a[1])
            ctxf_load(1, a2a[1][1])
            res10 = fin_wo(1, 0)
            fin_ln(1, 0, res10)
            res11 = fin_wo(1, 1)
            fin_ln(1, 1, res11)


def get_program(s=S, repeat=1):
    key = ("nc", s, repeat)
    if key not in _CACHE:
        _CACHE[key] = _build_program(s, repeat)
    return _CACHE[key]


def make_in_maps(hidden_states, attention_mask, Wq, bq, Wk, bk, Wv, bv, Wo, bo,
                 ln_gamma, ln_beta):
    """Host-side sharding: build the 8 per-core input maps."""
    bf = ml_dtypes.bfloat16
    hs = np.asarray(hidden_states, dtype=np.float32)
    b_, s_, h_ = hs.shape
    nkb = s_ // P
    rows = (b_ * s_) // NCORES
    rpb = rows // b_

    xT = np.ascontiguousarray(hs.transpose(0, 2, 1)).astype(bf)  # [B, H, S]
    Wq = np.asarray(Wq, np.float32)
    Wk = np.asarray(Wk, np.float32)
    Wv = np.asarray(Wv, np.float32)
    wo_f8 = np.ascontiguousarray(
        np.asarray(Wo, np.float32) * 256.0).astype(ml_dtypes.float8_e4m3)
    bq = np.asarray(bq, np.float32)
    bk = np.asarray(bk, np.float32)
    bv = np.asarray(bv, np.float32)
    bo = np.asarray(bo, np.float32)
    gamma_bc = np.ascontiguousarray(
        np.broadcast_to(np.asarray(ln_gamma, np.float32)[None, :], (P, H)))
    beta_bc = np.ascontiguousarray(
        np.broadcast_to(np.asarray(ln_beta, np.float32)[None, :], (P, H)))
    mask = np.asarray(attention_mask, np.float32).reshape(b_, s_)
    maskT = np.ascontiguousarray(
        mask.reshape(b_, nkb, P).transpose(0, 2, 1))  # [B, P, nkb]

    in_maps = []
    for c in range(NCORES):
        d0 = c * P
        rsl = slice(c * rpb, (c + 1) * rpb)
        xres_c = np.concatenate([hs[b, rsl, :] for b in range(b_)], axis=0)
        in_maps.append({
            "xT": xT,
            "wq": np.ascontiguousarray(Wq[:, d0:d0 + P]).astype(bf),
            "wk": np.ascontiguousarray(Wk[:, d0:d0 + P]).astype(bf),
            "wv": np.ascontiguousarray(Wv[:, d0:d0 + P]).astype(bf),
            "wo": wo_f8,
            "bq": np.ascontiguousarray(bq[d0:d0 + P]),
            "bk": np.ascontiguousarray(bk[d0:d0 + P]),
            "bv": np.ascontiguousarray(
                np.broadcast_to(bv[d0:d0 + P][None, :], (P, P))),
            "maskT": maskT,
            "xres": np.ascontiguousarray(
                (xres_c + bo[None, :]) * 65536.0),
            "gamma": gamma_bc,
            "beta": beta_bc,
        })
    return in_maps


def assemble_output(results, b_=B, s_=S, h_=H):
    rows = (b_ * s_) // NCORES
    rpb = rows // b_
    out = np.empty((b_, s_, h_), np.float32)
    for c in range(NCORES):
        r = np.asarray(results[c]["out"], np.float32)
        for b in range(b_):
            out[b, c * rpb:(c + 1) * rpb, :] = r[b * rpb:(b + 1) * rpb, :]
    return out


def kernel(**inputs):
    nc = get_program(S)
    in_maps = make_in_maps(**inputs)
    res = run_bass_kernel_spmd(nc, in_maps, list(range(NCORES)))
    return assemble_output(res.results)


# revision 10
# speedup vs baseline: 564.4168x; 1.3574x over previous
"""Trainium2 Bass kernel for BaseBertSelfAttention (B=2, S=2048, H=1024, 16 heads).

Sharding (8 NeuronCores):
  - Tensor-parallel on heads: core c owns heads (2c, 2c+1) -> d_local = 128.
  - Each core: QKV projections (column-parallel) for its 2 heads over BOTH
    batches, attention in transposed layout (scores^T: keys on partitions,
    queries on the free axis), softmax denominator via a ones-augmented V
    column, normalized context ctx^T [d_local=128, B*S].
  - Output rows are interleaved by batch: core c owns rows
    [c*256,(c+1)*256) of EACH batch.  This lets one 8-rank AllToAll per
    batch redistribute ctx^T from head-sharding to row-sharding; the
    batch-0 AllToAll and its Wo+LayerNorm tail fully overlap with the
    batch-1 QKV/attention compute, leaving only the (half-size) batch-1
    collective + tail exposed.
  - Each core then computes Wo projection + residual + LayerNorm for its
    2x128 output rows per batch.

Precision: bf16 matmul inputs (4x PE throughput), fp32 PSUM accumulation,
fp32 softmax denominators / reciprocal / residual / LayerNorm.  ctx and Wo
travel as scaled fp8 (the LayerNorm normalization cancels the scale).
"""

import numpy as np
import ml_dtypes

import concourse.bass as bass
import concourse.tile as tile
from concourse import bacc, mybir
from concourse.bass_utils import run_bass_kernel_spmd

BF16 = mybir.dt.bfloat16
FP8 = mybir.dt.float8e4
F32 = mybir.dt.float32
AF = mybir.ActivationFunctionType
ALU = mybir.AluOpType
P = 128

B, S, H = 2, 2048, 1024
NH, HD = 16, 64
NCORES = 8
EPS = 1e-12
SCALE = 1.0 / 8.0  # 1/sqrt(HD)
RPB = (B * S) // (NCORES * B)  # rows per (core, batch) = 256

_CACHE: dict = {}


def _build_program(s=S, repeat=1):
    """Build the (identical-across-cores) Bass program.

    repeat>1 replays the whole compute body that many times (same inputs,
    same output) -- used only by the timing harness to measure per-iteration
    device time with dispatch overhead amortized away.
    """
    nkb = s // P               # key blocks of 128
    qc_per_b = 4               # q chunks per batch (512 wide each)
    qw = s // qc_per_b         # 512
    rows = (B * s) // NCORES   # output rows per core (256 per batch)
    rpb = rows // B            # 256
    ho = H // P                # h chunks of 128 (8)

    nc = bacc.Bacc("TRN2", target_bir_lowering=False, debug=False,
                   num_devices=NCORES)
    xT = nc.dram_tensor("xT", [B, H, s], BF16, kind="ExternalInput")
    wq = nc.dram_tensor("wq", [H, P], BF16, kind="ExternalInput")
    wk = nc.dram_tensor("wk", [H, P], BF16, kind="ExternalInput")
    wv = nc.dram_tensor("wv", [H, P], BF16, kind="ExternalInput")
    wo = nc.dram_tensor("wo", [H, H], FP8, kind="ExternalInput")
    bq = nc.dram_tensor("bq", [P], F32, kind="ExternalInput")
    bk = nc.dram_tensor("bk", [P], F32, kind="ExternalInput")
    bv = nc.dram_tensor("bv", [P, P], F32, kind="ExternalInput")
    maskT = nc.dram_tensor("maskT", [B, P, nkb], F32, kind="ExternalInput")
    xres = nc.dram_tensor("xres", [rows, H], F32, kind="ExternalInput")
    gamma = nc.dram_tensor("gamma", [P, H], F32, kind="ExternalInput")
    beta = nc.dram_tensor("beta", [P, H], F32, kind="ExternalInput")
    out = nc.dram_tensor("out", [rows, H], F32, kind="ExternalOutput")

    with tile.TileContext(nc) as tc:
        _kernel_body(
            tc, s, nkb, qw, qc_per_b, rows, rpb, ho, repeat,
            xT, wq, wk, wv, wo, bq, bk, bv, maskT, xres, gamma, beta, out,
        )
    nc.compile()
    return nc


def _kernel_body(tc, s, nkb, qw, qc_per_b, rows, rpb, ho, repeat,
                 xT, wq, wk, wv, wo, bq, bk, bv, maskT, xres, gamma, beta, out):
    nc = tc.nc
    VPAD = 80  # padded free width of the ones-augmented V tiles (65 used)

    import contextlib
    stack = contextlib.ExitStack()
    with stack:
        consts = stack.enter_context(tc.tile_pool(name="consts", bufs=1))
        dram = stack.enter_context(tc.tile_pool(name="dram", bufs=2, space="DRAM"))

        # ---------------- constant / input loads ----------------
        wq_sb = consts.tile([P, ho, P], BF16)
        wk_sb = consts.tile([P, ho, P], BF16)
        wv_sb = consts.tile([P, ho, P], BF16)
        xT_sb = consts.tile([P, B, ho, s], BF16)
        xT_r = xT.rearrange("b (o p) s -> p b o s", p=P)

        # startup-critical order: wq, first x chunk, wk/wv, rest of x
        nc.sync.dma_start(wq_sb, wq.rearrange("(o p) d -> p o d", p=P))
        for o in range(ho):
            nc.sync.dma_start(xT_sb[:, 0, o, 0:512], xT_r[:, 0, o, 0:512])
        nc.sync.dma_start(wk_sb, wk.rearrange("(o p) d -> p o d", p=P))
        nc.sync.dma_start(wv_sb, wv.rearrange("(o p) d -> p o d", p=P))

        bq_sb = consts.tile([P, 1], F32)
        bk_sb = consts.tile([P, 1], F32)
        nc.sync.dma_start(bq_sb, bq[:, None])
        nc.sync.dma_start(bk_sb, bk[:, None])
        # host-prepared partition-broadcast tiles
        bv_b = consts.tile([P, P], F32)
        nc.sync.dma_start(bv_b, bv[:, :])
        mask_sb = consts.tile([P, B, nkb], F32)
        nc.sync.dma_start(mask_sb, maskT.rearrange("b p k -> p b k"))

        ones_sb = consts.tile([P, P], BF16)
        nc.vector.memset(ones_sb, 1.0)
        eps_sb = consts.tile([P, 1], F32)
        nc.vector.memset(eps_sb, EPS)

        # x^T (bf16): [p(h-inner), b, h-outer, s]; fine-grained loads in
        # consumption order (b, sc, o) so the first QKV chunk starts early
        for b in range(B):
            for sc in range(s // 512):
                if b == 0 and sc == 0:
                    continue
                sl = slice(sc * 512, (sc + 1) * 512)
                for o in range(ho):
                    nc.sync.dma_start(xT_sb[:, b, o, sl], xT_r[:, b, o, sl])

        wo_sb = consts.tile([P, ho, H], FP8)
        xres_sb = consts.tile([P, B, rpb // P, H], F32)
        gamma_b = consts.tile([P, H], F32)
        beta_b = consts.tile([P, H], F32)

        # attention intermediates
        qT_sb = consts.tile([P, B, s], BF16)   # Q^T [d_local, b, s]
        kT_sb = consts.tile([P, B, s], BF16)   # K^T [d_local, b, s]
        # ones-augmented V (natural layout), per head: [p(s-inner), b, kb, 65]
        v_e = consts.tile([P, B, nkb, VPAD], BF16)
        v_o = consts.tile([P, B, nkb, VPAD], BF16)
        nc.vector.memset(v_e, 1.0)
        nc.vector.memset(v_o, 1.0)
        # denominator column carries 1/256 so ctx*recip lands in fp8 range
        nc.vector.memset(v_e[:, :, :, 64:65], 1.0 / 256.0)
        nc.vector.memset(v_o[:, :, :, 64:65], 1.0 / 256.0)

        ctxf = consts.tile([P, B, ho, rpb], FP8)

        # PSUM: qk 2 banks (QKV drains + Wo + eb), s 2x2 banks, ctx 2 banks
        ps_qk = stack.enter_context(tc.tile_pool(name="ps_qk", bufs=2, space="PSUM"))
        ps_s = stack.enter_context(tc.tile_pool(name="ps_s", bufs=2, space="PSUM"))
        ps_ctx = stack.enter_context(tc.tile_pool(name="ps_ctx", bufs=2, space="PSUM"))
        ptile = stack.enter_context(tc.tile_pool(name="ptile", bufs=4))
        misc = stack.enter_context(tc.tile_pool(name="misc", bufs=3))
        fin = stack.enter_context(tc.tile_pool(name="fin", bufs=2))

        def qkv_proj(b, sc, w_sb, bias_sb, dst):
            sl = slice(sc * 512, (sc + 1) * 512)
            ps = ps_qk.tile([P, 512], F32, tag="qk")
            for o in range(ho):
                nc.tensor.matmul(
                    ps, lhsT=w_sb[:, o, :], rhs=xT_sb[:, b, o, sl],
                    start=(o == 0), stop=(o == ho - 1))
            nc.vector.tensor_tensor(
                dst[:, b, sl], ps,
                bias_sb[:, 0:1].to_broadcast((P, 512)),
                ALU.add)

        def qkv_vblock(b, kb):
            ksl = slice(kb * P, (kb + 1) * P)
            ps = ps_qk.tile([P, 512], F32, tag="qk")
            for o in range(ho):
                nc.tensor.matmul(
                    ps[:, 0:P], lhsT=xT_sb[:, b, o, ksl],
                    rhs=wv_sb[:, o, :],
                    start=(o == 0), stop=(o == ho - 1))
            nc.vector.tensor_tensor(
                v_e[:, b, kb, 0:64], ps[:, 0:64], bv_b[:, 0:64],
                ALU.add)
            nc.vector.tensor_tensor(
                v_o[:, b, kb, 0:64], ps[:, 64:128], bv_b[:, 64:128],
                ALU.add)

        def qkv_chunks(b):
            """QKV work split into ~1.7us PE chunks for interleaved emission."""
            kb_per_sc = nkb // (s // 512)
            for sc in range(s // 512):
                yield lambda sc=sc: qkv_proj(b, sc, wq_sb, bq_sb, qT_sb)
                yield lambda sc=sc: qkv_proj(b, sc, wk_sb, bk_sb, kT_sb)
                for kb in range(sc * kb_per_sc, (sc + 1) * kb_per_sc):
                    yield lambda kb=kb: qkv_vblock(b, kb)

        def qkv_stage(b):
            for ch in qkv_chunks(b):
                ch()

        def attn_qc(b, qc, a2a_in, filler=None):
            qsl = slice(qc * qw, (qc + 1) * qw)
            ctx_e = ps_ctx.tile([P, qw], F32, tag="ctx")
            ctx_o = ps_ctx.tile([P, qw], F32, tag="ctx")
            for kb in range(nkb):
                for f in (filler or ()):
                    f(qc * nkb + kb)
                ksl = slice(kb * P, (kb + 1) * P)
                sp = ps_s.tile([P, 2, qw], F32, tag="s")
                nc.tensor.matmul(
                    sp[:, 0, :], lhsT=kT_sb[0:64, b, ksl],
                    rhs=qT_sb[0:64, b, qsl], start=True, stop=True)
                nc.tensor.matmul(
                    sp[:, 1, :], lhsT=kT_sb[64:128, b, ksl],
                    rhs=qT_sb[64:128, b, qsl], start=True, stop=True)
                pp = ptile.tile([P, 2, qw], BF16, tag="p")
                nc.scalar.activation(
                    pp, sp, AF.Exp,
                    bias=mask_sb[:, b, kb:kb + 1], scale=SCALE)
                nc.tensor.matmul(
                    ctx_e[0:65, :], lhsT=v_e[:, b, kb, 0:65],
                    rhs=pp[:, 0, :], start=(kb == 0), stop=(kb == nkb - 1),
                    skip_group_check=True)
                nc.tensor.matmul(
                    ctx_o[0:65, :], lhsT=v_o[:, b, kb, 0:65],
                    rhs=pp[:, 1, :], start=(kb == 0), stop=(kb == nkb - 1),
                    skip_group_check=True)
            # normalize: ctx[d, q] / denom[q]  (denom = row 64), emit to
            # the two destination shards this q-chunk covers
            for h, ctx_ps in enumerate((ctx_e, ctx_o)):
                rinv = misc.tile([1, qw], F32, tag="rinv")
                nc.vector.reciprocal(rinv, ctx_ps[64:65, :])
                rb = misc.tile([1, qw], BF16, tag="rb")
                nc.vector.tensor_copy(out=rb, in_=rinv)
                eb = ps_qk.tile([64, qw], F32, tag="qk")
                nc.tensor.matmul(
                    eb, lhsT=ones_sb[0:1, 0:64], rhs=rb,
                    start=True, stop=True)
                ctx_bf = misc.tile([64, qw], BF16, tag="cb")
                nc.vector.tensor_copy(out=ctx_bf, in_=ctx_ps[0:64, :])
                cn = misc.tile([64, qw], FP8, tag="cn")
                nc.vector.tensor_tensor(cn, ctx_bf, eb, ALU.mult)
                for half in range(qw // rpb):
                    dest = qc * (qw // rpb) + half
                    r0 = dest * P + h * 64
                    nc.sync.dma_start(
                        a2a_in[r0:r0 + 64, :],
                        cn[:, half * rpb:(half + 1) * rpb])

        def fin_wo(b, qt):
            """Wo projection + residual for one 128-row tile -> res tile."""
            res = fin.tile([P, H], F32, tag="res")
            for nch in range(H // 512):
                nsl = slice(nch * 512, (nch + 1) * 512)
                ps = ps_qk.tile([P, 512], F32, tag="qk")
                for o in range(ho):
                    nc.tensor.matmul(
                        ps, lhsT=ctxf[:, b, o, qt * P:(qt + 1) * P],
                        rhs=wo_sb[:, o, nsl],
                        start=(o == 0), stop=(o == ho - 1))
                nc.vector.tensor_tensor(
                    res[:, nsl], ps, xres_sb[:, b, qt, nsl],
                    ALU.add)
            return res

        def fin_ln(b, qt, res):
            """LayerNorm over H (free axis) + store. DVE/Pool only -- the
            rstd is a Newton rsqrt so the ACT Exp table is never evicted."""
            stats = fin.tile([P, H // 512, 6], F32, tag="st")
            for g in range(H // 512):
                nc.vector.bn_stats(
                    stats[:, g, :], res[:, g * 512:(g + 1) * 512])
            mv = fin.tile([P, 2], F32, tag="mv")
            nc.vector.bn_aggr(out=mv, in_=stats)
            # y = rsqrt(var + eps) via Newton iterations (seed 1/65536:
            # rows are pre-scaled by 65536 and have ~unit variance)
            x = fin.tile([P, 1], F32, tag="x")
            nc.vector.tensor_tensor(x, mv[:, 1:2], eps_sb, ALU.add)
            y = fin.tile([P, 1], F32, tag="y")
            nc.vector.memset(y, 1.0 / 65536.0)
            t = fin.tile([P, 1], F32, tag="t")
            for _ in range(4):
                nc.vector.tensor_tensor(t, x, y, ALU.mult)
                nc.vector.tensor_tensor(t, t, y, ALU.mult)
                nc.vector.tensor_scalar(t, t, -0.5, 1.5, ALU.mult, ALU.add)
                nc.vector.tensor_tensor(y, y, t, ALU.mult)
            # (res - mu) * rstd in a single DVE pass
            nc.vector.tensor_scalar(
                res, res, mv[:, 0:1], y[:, 0:1],
                ALU.subtract, ALU.mult)
            outt = fin.tile([P, H], F32, tag="outt")
            nc.vector.tensor_tensor(outt, res, gamma_b, ALU.mult)
            nc.gpsimd.tensor_tensor(outt, outt, beta_b, ALU.add)
            nc.sync.dma_start(
                out[(b * (rpb // P) + qt) * P:(b * (rpb // P) + qt + 1) * P, :],
                outt)

        def do_a2a(pair):
            nc.gpsimd.collective_compute(
                "AllToAll", ALU.bypass,
                replica_groups=[list(range(NCORES))],
                ins=[pair[0][:].opt()], outs=[pair[1][:].opt()])

        def ctxf_load(b, a2a_out):
            for o in range(ho):
                nc.sync.dma_start(
                    ctxf[:, b, o, :],
                    a2a_out.rearrange("(o p) q -> p o q", p=P)[:, o, :])

        def make_filler(chunks, total_slots, start=0):
            """Spread chunk emission across attention kb slots >= start."""
            chunks = list(chunks)
            state = {"done": 0}
            n = len(chunks)

            def fill(g):
                if g < start:
                    return
                want = min(n, ((g - start + 1) * n) // max(1, total_slots - start))
                while state["done"] < want:
                    chunks[state["done"]]()
                    state["done"] += 1

            def flush():
                while state["done"] < len(chunks):
                    chunks[state["done"]]()
                    state["done"] += 1

            return fill, flush

        # preload the Exp activation table during the startup DMA wait
        warm = misc.tile([1, 1], F32, tag="warm")
        nc.scalar.activation(warm, eps_sb[0:1, :], AF.Exp)

        def fin_chunks(b):
            """The batch-b tail as filler chunks for the following phase."""
            resh = {}
            return [
                lambda: resh.__setitem__(0, fin_wo(b, 0)),
                lambda: fin_ln(b, 0, resh[0]),
                lambda: resh.__setitem__(1, fin_wo(b, 1)),
                lambda: fin_ln(b, 1, resh[1]),
            ]

        # software pipeline over 2*repeat phases: phase (it, b) runs batch-b
        # attention with (a) the next phase's QKV and (b) the previous
        # phase's Wo+LayerNorm tail interleaved into the kb loop, so the PE
        # stream never parks on a collective.
        nslots = qc_per_b * nkb
        qkv_stage(0)
        pending_fin = None
        for it in range(repeat):
            for b in range(B):
                a2a_i = dram.tile([NCORES * P, rpb], FP8, tag="a2ai",
                                  name=f"a2a_in_{it}_{b}")
                a2a_o = dram.tile([NCORES * P, rpb], FP8, tag="a2ao",
                                  name=f"a2a_out_{it}_{b}")
                last_phase = (it == repeat - 1) and (b == B - 1)
                fillers = []
                if not last_phase:
                    nb = (b + 1) % B
                    f_qkv, fl_qkv = make_filler(qkv_chunks(nb), nslots)
                    fillers.append(f_qkv)
                else:
                    fl_qkv = None
                if pending_fin is not None:
                    f_fin, fl_fin = make_filler(
                        pending_fin, nslots, start=int(nslots * 0.45))
                    fillers.append(f_fin)
                else:
                    fl_fin = None
                for qc in range(qc_per_b):
                    attn_qc(b, qc, a2a_i, filler=fillers)
                if fl_qkv:
                    fl_qkv()
                if fl_fin:
                    fl_fin()
                do_a2a((a2a_i, a2a_o))
                ctxf_load(b, a2a_o)
                if it == 0 and b == 0:
                    # tail-stage inputs (late emission => low DMA priority)
                    nc.sync.dma_start(
                        wo_sb, wo.rearrange("(o p) n -> p o n", p=P))
                    nc.sync.dma_start(
                        xres_sb,
                        xres.rearrange("(b r p) h -> p b r h", p=P, b=B))
                    nc.sync.dma_start(gamma_b, gamma[:, :])
                    nc.sync.dma_start(beta_b, beta[:, :])
                pending_fin = fin_chunks(b)
        for ch in pending_fin:
            ch()


def get_program(s=S, repeat=1):
    key = ("nc", s, repeat)
    if key not in _CACHE:
        _CACHE[key] = _build_program(s, repeat)
    return _CACHE[key]


def make_in_maps(hidden_states, attention_mask, Wq, bq, Wk, bk, Wv, bv, Wo, bo,
                 ln_gamma, ln_beta):
    """Host-side sharding: build the 8 per-core input maps."""
    bf = ml_dtypes.bfloat16
    hs = np.asarray(hidden_states, dtype=np.float32)
    b_, s_, h_ = hs.shape
    nkb = s_ // P
    rows = (b_ * s_) // NCORES
    rpb = rows // b_

    xT = np.ascontiguousarray(hs.transpose(0, 2, 1)).astype(bf)  # [B, H, S]
    Wq = np.asarray(Wq, np.float32)
    Wk = np.asarray(Wk, np.float32)
    Wv = np.asarray(Wv, np.float32)
    wo_f8 = np.ascontiguousarray(
        np.asarray(Wo, np.float32) * 256.0).astype(ml_dtypes.float8_e4m3)
    bq = np.asarray(bq, np.float32)
    bk = np.asarray(bk, np.float32)
    bv = np.asarray(bv, np.float32)
    bo = np.asarray(bo, np.float32)
    gamma_bc = np.ascontiguousarray(
        np.broadcast_to(np.asarray(ln_gamma, np.float32)[None, :], (P, H)))
    beta_bc = np.ascontiguousarray(
        np.broadcast_to(np.asarray(ln_beta, np.float32)[None, :], (P, H)))
    mask = np.asarray(attention_mask, np.float32).reshape(b_, s_)
    maskT = np.ascontiguousarray(
        mask.reshape(b_, nkb, P).transpose(0, 2, 1))  # [B, P, nkb]

    in_maps = []
    for c in range(NCORES):
        d0 = c * P
        rsl = slice(c * rpb, (c + 1) * rpb)
        xres_c = np.concatenate([hs[b, rsl, :] for b in range(b_)], axis=0)
        in_maps.append({
            "xT": xT,
            "wq": np.ascontiguousarray(Wq[:, d0:d0 + P]).astype(bf),
            "wk": np.ascontiguousarray(Wk[:, d0:d0 + P]).astype(bf),
            "wv": np.ascontiguousarray(Wv[:, d0:d0 + P]).astype(bf),
            "wo": wo_f8,
            "bq": np.ascontiguousarray(bq[d0:d0 + P]),
            "bk": np.ascontiguousarray(bk[d0:d0 + P]),
            "bv": np.ascontiguousarray(
                np.broadcast_to(bv[d0:d0 + P][None, :], (P, P))),
            "maskT": maskT,
            "xres": np.ascontiguousarray(
                (xres_c + bo[None, :]) * 65536.0),
            "gamma": gamma_bc,
            "beta": beta_bc,
        })
    return in_maps


def assemble_output(results, b_=B, s_=S, h_=H):
    rows = (b_ * s_) // NCORES
    rpb = rows // b_
    out = np.empty((b_, s_, h_), np.float32)
    for c in range(NCORES):
        r = np.asarray(results[c]["out"], np.float32)
        for b in range(b_):
            out[b, c * rpb:(c + 1) * rpb, :] = r[b * rpb:(b + 1) * rpb, :]
    return out


def kernel(**inputs):
    nc = get_program(S)
    in_maps = make_in_maps(**inputs)
    res = run_bass_kernel_spmd(nc, in_maps, list(range(NCORES)))
    return assemble_output(res.results)


# revision 13
# speedup vs baseline: 682.7499x; 1.2097x over previous
"""Trainium2 Bass kernel for BaseBertSelfAttention (B=2, S=2048, H=1024, 16 heads).

Sharding (8 NeuronCores):
  - Tensor-parallel on heads: core c owns heads (2c, 2c+1) -> d_local = 128.
  - Each core: QKV projections (column-parallel) for its 2 heads over BOTH
    batches, attention in transposed layout (scores^T: keys on partitions,
    queries on the free axis), softmax denominator via a ones-augmented V
    column, normalized context ctx^T [d_local=128, B*S].
  - Output rows are interleaved by batch: core c owns rows
    [c*256,(c+1)*256) of EACH batch.  This lets one 8-rank AllToAll per
    batch redistribute ctx^T from head-sharding to row-sharding; the
    batch-0 AllToAll and its Wo+LayerNorm tail fully overlap with the
    batch-1 QKV/attention compute, leaving only the (half-size) batch-1
    collective + tail exposed.
  - Each core then computes Wo projection + residual + LayerNorm for its
    2x128 output rows per batch.

Precision: bf16 matmul inputs (4x PE throughput), fp32 PSUM accumulation,
fp32 softmax denominators / reciprocal / residual / LayerNorm.  ctx and Wo
travel as scaled fp8 (the LayerNorm normalization cancels the scale).
"""

import numpy as np
import ml_dtypes

import concourse.bass as bass
import concourse.tile as tile
from concourse import bacc, mybir
from concourse.bass_utils import run_bass_kernel_spmd

BF16 = mybir.dt.bfloat16
FP8 = mybir.dt.float8e4
F32 = mybir.dt.float32
AF = mybir.ActivationFunctionType
ALU = mybir.AluOpType
P = 128

B, S, H = 2, 2048, 1024
NH, HD = 16, 64
NCORES = 8
EPS = 1e-12
SCALE = 1.0 / 8.0  # 1/sqrt(HD)
LN256 = float(np.log(256.0))
RPB = (B * S) // (NCORES * B)  # rows per (core, batch) = 256

_CACHE: dict = {}


def _build_program(s=S, repeat=1, zero_mask=True):
    """Build the (identical-across-cores) Bass program.

    repeat>1 replays the whole compute body that many times (same inputs,
    same output) -- used only by the timing harness to measure per-iteration
    device time with dispatch overhead amortized away.
    """
    nkb = s // P               # key blocks of 128
    qc_per_b = 4               # q chunks per batch (512 wide each)
    qw = s // qc_per_b         # 512
    rows = (B * s) // NCORES   # output rows per core (256 per batch)
    rpb = rows // B            # 256
    ho = H // P                # h chunks of 128 (8)

    nc = bacc.Bacc("TRN2", target_bir_lowering=False, debug=False,
                   num_devices=NCORES)
    xT = nc.dram_tensor("xT", [B, H, s], BF16, kind="ExternalInput")
    wq = nc.dram_tensor("wq", [H, P], BF16, kind="ExternalInput")
    wk = nc.dram_tensor("wk", [H, P], BF16, kind="ExternalInput")
    wv = nc.dram_tensor("wv", [H, P], BF16, kind="ExternalInput")
    wo = nc.dram_tensor("wo", [H, H], FP8, kind="ExternalInput")
    bq = nc.dram_tensor("bq", [P], F32, kind="ExternalInput")
    bk = nc.dram_tensor("bk", [P], F32, kind="ExternalInput")
    bv = nc.dram_tensor("bv", [P, P], F32, kind="ExternalInput")
    maskT = nc.dram_tensor("maskT", [B, P, nkb], F32, kind="ExternalInput")
    xres = nc.dram_tensor("xres", [rows, H], F32, kind="ExternalInput")
    gamma = nc.dram_tensor("gamma", [P, H], F32, kind="ExternalInput")
    beta = nc.dram_tensor("beta", [P, H], F32, kind="ExternalInput")
    out = nc.dram_tensor("out", [rows, H], F32, kind="ExternalOutput")

    with tile.TileContext(nc) as tc:
        _kernel_body(
            tc, s, nkb, qw, qc_per_b, rows, rpb, ho, repeat, zero_mask,
            xT, wq, wk, wv, wo, bq, bk, bv, maskT, xres, gamma, beta, out,
        )
    nc.compile()
    return nc


def _kernel_body(tc, s, nkb, qw, qc_per_b, rows, rpb, ho, repeat, zero_mask,
                 xT, wq, wk, wv, wo, bq, bk, bv, maskT, xres, gamma, beta, out):
    nc = tc.nc
    VPAD = 80  # padded free width of the ones-augmented V tiles (65 used)

    import contextlib
    stack = contextlib.ExitStack()
    with stack:
        consts = stack.enter_context(tc.tile_pool(name="consts", bufs=1))
        dram = stack.enter_context(tc.tile_pool(name="dram", bufs=2, space="DRAM"))

        # ---------------- constant / input loads ----------------
        wq_sb = consts.tile([P, ho, P], BF16)
        wk_sb = consts.tile([P, ho, P], BF16)
        wv_sb = consts.tile([P, ho, P], BF16)
        xT_sb = consts.tile([P, B, ho, s], BF16)
        xT_r = xT.rearrange("b (o p) s -> p b o s", p=P)

        # startup-critical order: wq, first x chunk, wk/wv, rest of x
        nc.sync.dma_start(wq_sb, wq.rearrange("(o p) d -> p o d", p=P))
        for o in range(ho):
            nc.sync.dma_start(xT_sb[:, 0, o, 0:512], xT_r[:, 0, o, 0:512])
        nc.sync.dma_start(wk_sb, wk.rearrange("(o p) d -> p o d", p=P))
        nc.sync.dma_start(wv_sb, wv.rearrange("(o p) d -> p o d", p=P))

        bq_sb = consts.tile([P, 1], F32)
        bk_sb = consts.tile([P, 1], F32)
        nc.sync.dma_start(bq_sb, bq[:, None])
        nc.sync.dma_start(bk_sb, bk[:, None])
        # host-prepared partition-broadcast tiles
        bv_b = consts.tile([P, P], F32)
        nc.sync.dma_start(bv_b, bv[:, :])
        mask_sb = consts.tile([P, B, nkb], F32)
        nc.sync.dma_start(mask_sb, maskT.rearrange("b p k -> p b k"))

        ones_sb = consts.tile([P, P], BF16)
        nc.vector.memset(ones_sb, 1.0)
        eps_sb = consts.tile([P, 1], F32)
        nc.vector.memset(eps_sb, EPS)
        zero_sb = consts.tile([P, 1], F32)
        nc.vector.memset(zero_sb, 0.0)

        # x^T (bf16): [p(h-inner), b, h-outer, s]; fine-grained loads in
        # consumption order (b, sc, o) so the first QKV chunk starts early
        for b in range(B):
            for sc in range(s // 512):
                if b == 0 and sc == 0:
                    continue
                sl = slice(sc * 512, (sc + 1) * 512)
                for o in range(ho):
                    nc.sync.dma_start(xT_sb[:, b, o, sl], xT_r[:, b, o, sl])

        wo_sb = consts.tile([P, ho, H], FP8)
        xres_sb = consts.tile([P, B, rpb // P, H], F32)
        gamma_b = consts.tile([P, H], F32)
        beta_b = consts.tile([P, H], F32)

        # attention intermediates
        qT_sb = consts.tile([P, B, s], BF16)   # Q^T [d_local, b, s]
        kT_sb = consts.tile([P, B, s], BF16)   # K^T [d_local, b, s]
        # ones-augmented V (natural layout), per head: [p(s-inner), b, kb, 65]
        v_e = consts.tile([P, B, nkb, VPAD], FP8)
        v_o = consts.tile([P, B, nkb, VPAD], FP8)
        nc.vector.memset(v_e, 1.0)
        nc.vector.memset(v_o, 1.0)

        ctxf = consts.tile([P, B, ho, rpb], FP8)

        # PSUM: qk 2 banks (QKV drains + Wo + eb), s 2x2 banks, ctx 2 banks
        ps_qk = stack.enter_context(tc.tile_pool(name="ps_qk", bufs=2, space="PSUM"))
        ps_s = stack.enter_context(tc.tile_pool(name="ps_s", bufs=2, space="PSUM"))
        ps_ctx = stack.enter_context(tc.tile_pool(name="ps_ctx", bufs=2, space="PSUM"))
        ptile = stack.enter_context(tc.tile_pool(name="ptile", bufs=4))
        misc = stack.enter_context(tc.tile_pool(name="misc", bufs=3))
        fin = stack.enter_context(tc.tile_pool(name="fin", bufs=2))

        def qkv_proj(b, sc, w_sb, bias_sb, dst):
            sl = slice(sc * 512, (sc + 1) * 512)
            ps = ps_qk.tile([P, 512], F32, tag="qk")
            for o in range(ho):
                nc.tensor.matmul(
                    ps, lhsT=w_sb[:, o, :], rhs=xT_sb[:, b, o, sl],
                    start=(o == 0), stop=(o == ho - 1))
            nc.vector.tensor_tensor(
                dst[:, b, sl], ps,
                bias_sb[:, 0:1].to_broadcast((P, 512)),
                ALU.add)

        def qkv_vblock(b, kb):
            ksl = slice(kb * P, (kb + 1) * P)
            ps = ps_qk.tile([P, 512], F32, tag="qk")
            for o in range(ho):
                nc.tensor.matmul(
                    ps[:, 0:P], lhsT=xT_sb[:, b, o, ksl],
                    rhs=wv_sb[:, o, :],
                    start=(o == 0), stop=(o == ho - 1))
            nc.vector.tensor_tensor(
                v_e[:, b, kb, 0:64], ps[:, 0:64], bv_b[:, 0:64],
                ALU.add)
            nc.vector.tensor_tensor(
                v_o[:, b, kb, 0:64], ps[:, 64:128], bv_b[:, 64:128],
                ALU.add)

        def qkv_chunks(b):
            """QKV work split into ~1.7us PE chunks for interleaved emission."""
            kb_per_sc = nkb // (s // 512)
            for sc in range(s // 512):
                yield lambda sc=sc: qkv_proj(b, sc, wq_sb, bq_sb, qT_sb)
                yield lambda sc=sc: qkv_proj(b, sc, wk_sb, bk_sb, kT_sb)
                for kb in range(sc * kb_per_sc, (sc + 1) * kb_per_sc):
                    yield lambda kb=kb: qkv_vblock(b, kb)

        def qkv_stage(b):
            for ch in qkv_chunks(b):
                ch()

        def attn_qc(b, qc, a2a_in, filler=None):
            qsl = slice(qc * qw, (qc + 1) * qw)
            ctx_e = ps_ctx.tile([P, qw], F32, tag="ctx")
            ctx_o = ps_ctx.tile([P, qw], F32, tag="ctx")
            npair = nkb // 2
            for pr in range(npair):
                kb0, kb1 = 2 * pr, 2 * pr + 1
                for f in (filler or ()):
                    f(qc * npair * 2 + 2 * pr)
                sl0 = slice(kb0 * P, (kb0 + 1) * P)
                sl1 = slice(kb1 * P, (kb1 + 1) * P)
                # scores for both key blocks of the pair, per head; the two
                # heads auto-pack into row groups (0,0)/(64,0) of the array
                sp_e = ps_s.tile([P, 2, qw], F32, tag="s")
                sp_o = ps_s.tile([P, 2, qw], F32, tag="s")
                nc.tensor.matmul(
                    sp_e[:, 0, :], lhsT=kT_sb[0:64, b, sl0],
                    rhs=qT_sb[0:64, b, qsl], start=True, stop=True)
                nc.tensor.matmul(
                    sp_o[:, 0, :], lhsT=kT_sb[64:128, b, sl0],
                    rhs=qT_sb[64:128, b, qsl], start=True, stop=True)
                nc.tensor.matmul(
                    sp_e[:, 1, :], lhsT=kT_sb[0:64, b, sl1],
                    rhs=qT_sb[0:64, b, qsl], start=True, stop=True)
                nc.tensor.matmul(
                    sp_o[:, 1, :], lhsT=kT_sb[64:128, b, sl1],
                    rhs=qT_sb[64:128, b, qsl], start=True, stop=True)
                # exp of the raw logits: unnormalized exp(s/8) is O(1) and
                # sits comfortably in fp8e4m3's normal range
                pp_e = ptile.tile([P, 2, qw], FP8, tag="p")
                pp_o = ptile.tile([P, 2, qw], FP8, tag="p")
                if zero_mask:
                    nc.scalar.activation(pp_e, sp_e, AF.Exp,
                                         bias=zero_sb[:, 0:1], scale=SCALE)
                    nc.scalar.activation(pp_o, sp_o, AF.Exp,
                                         bias=zero_sb[:, 0:1], scale=SCALE)
                else:
                    for t, kb in ((0, kb0), (1, kb1)):
                        nc.scalar.activation(
                            pp_e[:, t, :], sp_e[:, t, :], AF.Exp,
                            bias=mask_sb[:, b, kb:kb + 1], scale=SCALE)
                        nc.scalar.activation(
                            pp_o[:, t, :], sp_o[:, t, :], AF.Exp,
                            bias=mask_sb[:, b, kb:kb + 1], scale=SCALE)
                # fp8 DoubleRow: both key blocks in one K=256 matmul
                nc.tensor.matmul(
                    ctx_e[0:65, :], lhsT=v_e[:, b, kb0:kb0 + 2, 0:65],
                    rhs=pp_e, start=(pr == 0), stop=(pr == npair - 1),
                    perf_mode=mybir.MatmulPerfMode.DoubleRow,
                    skip_group_check=True)
                nc.tensor.matmul(
                    ctx_o[0:65, :], lhsT=v_o[:, b, kb0:kb0 + 2, 0:65],
                    rhs=pp_o, start=(pr == 0), stop=(pr == npair - 1),
                    perf_mode=mybir.MatmulPerfMode.DoubleRow,
                    skip_group_check=True)
            # normalize: ctx[d, q] / denom[q]  (denom = row 64), emit to
            # the two destination shards this q-chunk covers
            for h, ctx_ps in enumerate((ctx_e, ctx_o)):
                rinv = misc.tile([1, qw], F32, tag="rinv")
                nc.vector.reciprocal(rinv, ctx_ps[64:65, :])
                rb = misc.tile([1, qw], BF16, tag="rb")
                nc.vector.tensor_scalar(rb, rinv, 256.0, None, ALU.mult)
                eb = ps_qk.tile([64, qw], F32, tag="qk")
                nc.tensor.matmul(
                    eb, lhsT=ones_sb[0:1, 0:64], rhs=rb,
                    start=True, stop=True)
                ctx_bf = misc.tile([64, qw], BF16, tag="cb")
                nc.vector.tensor_copy(out=ctx_bf, in_=ctx_ps[0:64, :])
                cn = misc.tile([64, qw], FP8, tag="cn")
                nc.vector.tensor_tensor(cn, ctx_bf, eb, ALU.mult)
                for half in range(qw // rpb):
                    dest = qc * (qw // rpb) + half
                    r0 = dest * P + h * 64
                    nc.sync.dma_start(
                        a2a_in[r0:r0 + 64, :],
                        cn[:, half * rpb:(half + 1) * rpb])

        def fin_wo(b, qt):
            """Wo projection + residual for one 128-row tile -> res tile."""
            res = fin.tile([P, H], F32, tag="res")
            for nch in range(H // 512):
                nsl = slice(nch * 512, (nch + 1) * 512)
                ps = ps_qk.tile([P, 512], F32, tag="qk")
                for o in range(ho):
                    nc.tensor.matmul(
                        ps, lhsT=ctxf[:, b, o, qt * P:(qt + 1) * P],
                        rhs=wo_sb[:, o, nsl],
                        start=(o == 0), stop=(o == ho - 1))
                nc.vector.tensor_tensor(
                    res[:, nsl], ps, xres_sb[:, b, qt, nsl],
                    ALU.add)
            return res

        def fin_ln(b, qt, res):
            """LayerNorm over H (free axis) + store. DVE/Pool only -- the
            rstd is a Newton rsqrt so the ACT Exp table is never evicted."""
            stats = fin.tile([P, H // 512, 6], F32, tag="st")
            for g in range(H // 512):
                nc.vector.bn_stats(
                    stats[:, g, :], res[:, g * 512:(g + 1) * 512])
            mv = fin.tile([P, 2], F32, tag="mv")
            nc.vector.bn_aggr(out=mv, in_=stats)
            # y = rsqrt(var + eps) via Newton iterations (seed 1/65536:
            # rows are pre-scaled by 65536 and have ~unit variance)
            x = fin.tile([P, 1], F32, tag="x")
            nc.vector.tensor_tensor(x, mv[:, 1:2], eps_sb, ALU.add)
            y = fin.tile([P, 1], F32, tag="y")
            nc.vector.memset(y, 1.0 / 65536.0)
            t = fin.tile([P, 1], F32, tag="t")
            for _ in range(4):
                nc.vector.tensor_tensor(t, x, y, ALU.mult)
                nc.vector.tensor_tensor(t, t, y, ALU.mult)
                nc.vector.tensor_scalar(t, t, -0.5, 1.5, ALU.mult, ALU.add)
                nc.vector.tensor_tensor(y, y, t, ALU.mult)
            # (res - mu) * rstd in a single DVE pass
            nc.vector.tensor_scalar(
                res, res, mv[:, 0:1], y[:, 0:1],
                ALU.subtract, ALU.mult)
            outt = fin.tile([P, H], F32, tag="outt")
            nc.vector.tensor_tensor(outt, res, gamma_b, ALU.mult)
            nc.gpsimd.tensor_tensor(outt, outt, beta_b, ALU.add)
            nc.sync.dma_start(
                out[(b * (rpb // P) + qt) * P:(b * (rpb // P) + qt + 1) * P, :],
                outt)

        def do_a2a(pair):
            nc.gpsimd.collective_compute(
                "AllToAll", ALU.bypass,
                replica_groups=[list(range(NCORES))],
                ins=[pair[0][:].opt()], outs=[pair[1][:].opt()])

        def ctxf_load(b, a2a_out):
            for o in range(ho):
                nc.sync.dma_start(
                    ctxf[:, b, o, :],
                    a2a_out.rearrange("(o p) q -> p o q", p=P)[:, o, :])

        def make_filler(chunks, total_slots, start=0):
            """Spread chunk emission across attention kb slots >= start."""
            chunks = list(chunks)
            state = {"done": 0}
            n = len(chunks)

            def fill(g):
                if g < start:
                    return
                want = min(n, ((g - start + 1) * n) // max(1, total_slots - start))
                while state["done"] < want:
                    chunks[state["done"]]()
                    state["done"] += 1

            def flush():
                while state["done"] < len(chunks):
                    chunks[state["done"]]()
                    state["done"] += 1

            return fill, flush

        # preload the Exp activation table during the startup DMA wait
        warm = misc.tile([1, 1], F32, tag="warm")
        nc.scalar.activation(warm, eps_sb[0:1, :], AF.Exp)

        def fin_chunks(b):
            """The batch-b tail as filler chunks for the following phase."""
            resh = {}
            return [
                lambda: resh.__setitem__(0, fin_wo(b, 0)),
                lambda: fin_ln(b, 0, resh[0]),
                lambda: resh.__setitem__(1, fin_wo(b, 1)),
                lambda: fin_ln(b, 1, resh[1]),
            ]

        # software pipeline over 2*repeat phases: phase (it, b) runs batch-b
        # attention with (a) the next phase's QKV and (b) the previous
        # phase's Wo+LayerNorm tail interleaved into the kb loop, so the PE
        # stream never parks on a collective.
        nslots = qc_per_b * nkb
        qkv_stage(0)
        pending_fin = None
        for it in range(repeat):
            for b in range(B):
                a2a_i = dram.tile([NCORES * P, rpb], FP8, tag="a2ai",
                                  name=f"a2a_in_{it}_{b}")
                a2a_o = dram.tile([NCORES * P, rpb], FP8, tag="a2ao",
                                  name=f"a2a_out_{it}_{b}")
                last_phase = (it == repeat - 1) and (b == B - 1)
                fillers = []
                if not last_phase:
                    nb = (b + 1) % B
                    f_qkv, fl_qkv = make_filler(qkv_chunks(nb), nslots)
                    fillers.append(f_qkv)
                else:
                    fl_qkv = None
                if pending_fin is not None:
                    f_fin, fl_fin = make_filler(
                        pending_fin, nslots, start=int(nslots * 0.45))
                    fillers.append(f_fin)
                else:
                    fl_fin = None
                for qc in range(qc_per_b):
                    attn_qc(b, qc, a2a_i, filler=fillers)
                if fl_qkv:
                    fl_qkv()
                if fl_fin:
                    fl_fin()
                do_a2a((a2a_i, a2a_o))
                ctxf_load(b, a2a_o)
                if it == 0 and b == 0:
                    # tail-stage inputs (late emission => low DMA priority)
                    nc.sync.dma_start(
                        wo_sb, wo.rearrange("(o p) n -> p o n", p=P))
                    nc.sync.dma_start(
                        xres_sb,
                        xres.rearrange("(b r p) h -> p b r h", p=P, b=B))
                    nc.sync.dma_start(gamma_b, gamma[:, :])
                    nc.sync.dma_start(beta_b, beta[:, :])
                pending_fin = fin_chunks(b)
        for ch in pending_fin:
            ch()


def get_program(s=S, repeat=1, zero_mask=True):
    key = ("nc", s, repeat, zero_mask)
    if key not in _CACHE:
        _CACHE[key] = _build_program(s, repeat, zero_mask)
    return _CACHE[key]


def make_in_maps(hidden_states, attention_mask, Wq, bq, Wk, bk, Wv, bv, Wo, bo,
                 ln_gamma, ln_beta):
    """Host-side sharding: build the 8 per-core input maps."""
    bf = ml_dtypes.bfloat16
    hs = np.asarray(hidden_states, dtype=np.float32)
    b_, s_, h_ = hs.shape
    nkb = s_ // P
    rows = (b_ * s_) // NCORES
    rpb = rows // b_

    xT = np.ascontiguousarray(hs.transpose(0, 2, 1)).astype(bf)  # [B, H, S]
    Wq = np.asarray(Wq, np.float32)
    Wk = np.asarray(Wk, np.float32)
    Wv = np.asarray(Wv, np.float32)
    wo_f8 = np.ascontiguousarray(
        np.asarray(Wo, np.float32) * 256.0).astype(ml_dtypes.float8_e4m3)
    bq = np.asarray(bq, np.float32)
    bk = np.asarray(bk, np.float32)
    bv = np.asarray(bv, np.float32)
    bo = np.asarray(bo, np.float32)
    gamma_bc = np.ascontiguousarray(
        np.broadcast_to(np.asarray(ln_gamma, np.float32)[None, :], (P, H)))
    beta_bc = np.ascontiguousarray(
        np.broadcast_to(np.asarray(ln_beta, np.float32)[None, :], (P, H)))
    mask = np.asarray(attention_mask, np.float32).reshape(b_, s_)
    maskT = np.ascontiguousarray(
        mask.reshape(b_, nkb, P).transpose(0, 2, 1))  # [B, P, nkb]

    in_maps = []
    for c in range(NCORES):
        d0 = c * P
        rsl = slice(c * rpb, (c + 1) * rpb)
        xres_c = np.concatenate([hs[b, rsl, :] for b in range(b_)], axis=0)
        in_maps.append({
            "xT": xT,
            "wq": np.ascontiguousarray(Wq[:, d0:d0 + P]).astype(bf),
            "wk": np.ascontiguousarray(Wk[:, d0:d0 + P]).astype(bf),
            "wv": np.ascontiguousarray(Wv[:, d0:d0 + P]).astype(bf),
            "wo": wo_f8,
            "bq": np.ascontiguousarray(bq[d0:d0 + P]),
            "bk": np.ascontiguousarray(bk[d0:d0 + P]),
            "bv": np.ascontiguousarray(
                np.broadcast_to(bv[d0:d0 + P][None, :], (P, P))),
            "maskT": maskT,
            "xres": np.ascontiguousarray(
                (xres_c + bo[None, :]) * 65536.0),
            "gamma": gamma_bc,
            "beta": beta_bc,
        })
    return in_maps


def assemble_output(results, b_=B, s_=S, h_=H):
    rows = (b_ * s_) // NCORES
    rpb = rows // b_
    out = np.empty((b_, s_, h_), np.float32)
    for c in range(NCORES):
        r = np.asarray(results[c]["out"], np.float32)
        for b in range(b_):
            out[b, c * rpb:(c + 1) * rpb, :] = r[b * rpb:(b + 1) * rpb, :]
    return out


def kernel(**inputs):
    zero_mask = bool(np.all(np.asarray(inputs["attention_mask"]) == 0.0))
    nc = get_program(S, zero_mask=zero_mask)
    in_maps = make_in_maps(**inputs)
    res = run_bass_kernel_spmd(nc, in_maps, list(range(NCORES)))
    return assemble_output(res.results)


# revision 14
# speedup vs baseline: 895.4916x; 1.3116x over previous
"""Trainium2 Bass kernel for BaseBertSelfAttention (B=2, S=2048, H=1024, 16 heads).

Sharding (8 NeuronCores):
  - Tensor-parallel on heads: core c owns heads (2c, 2c+1) -> d_local = 128.
  - Each core: QKV projections (column-parallel) for its 2 heads over BOTH
    batches, attention in transposed layout (scores^T: keys on partitions,
    queries on the free axis), softmax denominator via a ones-augmented V
    column, normalized context ctx^T [d_local=128, B*S].
  - Output rows are interleaved by batch: core c owns rows
    [c*256,(c+1)*256) of EACH batch.  This lets one 8-rank AllToAll per
    batch redistribute ctx^T from head-sharding to row-sharding; the
    batch-0 AllToAll and its Wo+LayerNorm tail fully overlap with the
    batch-1 QKV/attention compute, leaving only the (half-size) batch-1
    collective + tail exposed.
  - Each core then computes Wo projection + residual + LayerNorm for its
    2x128 output rows per batch.

Precision: bf16 matmul inputs (4x PE throughput), fp32 PSUM accumulation,
fp32 softmax denominators / reciprocal / residual / LayerNorm.  ctx and Wo
travel as scaled fp8 (the LayerNorm normalization cancels the scale).
"""

import numpy as np
import ml_dtypes

import concourse.bass as bass
import concourse.tile as tile
from concourse import bacc, mybir
from concourse.bass_utils import run_bass_kernel_spmd

BF16 = mybir.dt.bfloat16
FP8 = mybir.dt.float8e4
F32 = mybir.dt.float32
AF = mybir.ActivationFunctionType
ALU = mybir.AluOpType
P = 128

B, S, H = 2, 2048, 1024
NH, HD = 16, 64
NCORES = 8
EPS = 1e-12
SCALE = 1.0 / 8.0  # 1/sqrt(HD)
LN256 = float(np.log(256.0))
RPB = (B * S) // (NCORES * B)  # rows per (core, batch) = 256

_CACHE: dict = {}


def _build_program(s=S, repeat=1, zero_mask=True):
    """Build the (identical-across-cores) Bass program.

    repeat>1 replays the whole compute body that many times (same inputs,
    same output) -- used only by the timing harness to measure per-iteration
    device time with dispatch overhead amortized away.
    """
    nkb = s // P               # key blocks of 128
    qc_per_b = 4               # q chunks per batch (512 wide each)
    qw = s // qc_per_b         # 512
    rows = (B * s) // NCORES   # output rows per core (256 per batch)
    rpb = rows // B            # 256
    ho = H // P                # h chunks of 128 (8)

    nc = bacc.Bacc("TRN2", target_bir_lowering=False, debug=False,
                   num_devices=NCORES)
    xT = nc.dram_tensor("xT", [B, H, s], BF16, kind="ExternalInput")
    wq = nc.dram_tensor("wq", [H, P], BF16, kind="ExternalInput")
    wk = nc.dram_tensor("wk", [H, P], BF16, kind="ExternalInput")
    wv = nc.dram_tensor("wv", [H, P], BF16, kind="ExternalInput")
    wo = nc.dram_tensor("wo", [H, H], FP8, kind="ExternalInput")
    bq = nc.dram_tensor("bq", [P], F32, kind="ExternalInput")
    bk = nc.dram_tensor("bk", [P], F32, kind="ExternalInput")
    bv = nc.dram_tensor("bv", [P, P], F32, kind="ExternalInput")
    maskT = nc.dram_tensor("maskT", [B, P, nkb], F32, kind="ExternalInput")
    xres = nc.dram_tensor("xres", [rows, H], F32, kind="ExternalInput")
    gamma = nc.dram_tensor("gamma", [P, H], F32, kind="ExternalInput")
    beta = nc.dram_tensor("beta", [P, H], F32, kind="ExternalInput")
    out = nc.dram_tensor("out", [rows, H], F32, kind="ExternalOutput")

    with tile.TileContext(nc) as tc:
        _kernel_body(
            tc, s, nkb, qw, qc_per_b, rows, rpb, ho, repeat, zero_mask,
            xT, wq, wk, wv, wo, bq, bk, bv, maskT, xres, gamma, beta, out,
        )
    nc.compile()
    return nc


def _kernel_body(tc, s, nkb, qw, qc_per_b, rows, rpb, ho, repeat, zero_mask,
                 xT, wq, wk, wv, wo, bq, bk, bv, maskT, xres, gamma, beta, out):
    nc = tc.nc
    VPAD = 80  # padded free width of the ones-augmented V tiles (65 used)

    import contextlib
    stack = contextlib.ExitStack()
    with stack:
        consts = stack.enter_context(tc.tile_pool(name="consts", bufs=1))
        dram = stack.enter_context(tc.tile_pool(name="dram", bufs=2, space="DRAM"))

        # ---------------- constant / input loads ----------------
        wq_sb = consts.tile([P, ho, P], BF16)
        wk_sb = consts.tile([P, ho, P], BF16)
        wv_sb = consts.tile([P, ho, P], BF16)
        xT_sb = consts.tile([P, B, ho, s], BF16)
        xT_r = xT.rearrange("b (o p) s -> p b o s", p=P)

        # startup-critical order: wq, first x chunk, wk/wv, rest of x
        nc.sync.dma_start(wq_sb, wq.rearrange("(o p) d -> p o d", p=P))
        for o in range(ho):
            nc.sync.dma_start(xT_sb[:, 0, o, 0:512], xT_r[:, 0, o, 0:512])
        nc.sync.dma_start(wk_sb, wk.rearrange("(o p) d -> p o d", p=P))
        nc.sync.dma_start(wv_sb, wv.rearrange("(o p) d -> p o d", p=P))

        bq_sb = consts.tile([P, 1], F32)
        bk_sb = consts.tile([P, 1], F32)
        nc.sync.dma_start(bq_sb, bq[:, None])
        nc.sync.dma_start(bk_sb, bk[:, None])
        # host-prepared partition-broadcast tiles
        bv_b = consts.tile([P, P], F32)
        nc.sync.dma_start(bv_b, bv[:, :])
        mask_sb = consts.tile([P, B, nkb], F32)
        nc.sync.dma_start(mask_sb, maskT.rearrange("b p k -> p b k"))

        ones_sb = consts.tile([P, P], BF16)
        nc.vector.memset(ones_sb, 1.0)
        eps_sb = consts.tile([P, 1], F32)
        nc.vector.memset(eps_sb, EPS)
        zero_sb = consts.tile([P, 1], F32)
        nc.vector.memset(zero_sb, 0.0)

        # x^T (bf16): [p(h-inner), b, h-outer, s]; fine-grained loads in
        # consumption order (b, sc, o) so the first QKV chunk starts early
        for b in range(B):
            for sc in range(s // 512):
                if b == 0 and sc == 0:
                    continue
                sl = slice(sc * 512, (sc + 1) * 512)
                for o in range(ho):
                    nc.sync.dma_start(xT_sb[:, b, o, sl], xT_r[:, b, o, sl])

        wo_sb = consts.tile([P, ho, H], FP8)
        xres_sb = consts.tile([P, B, rpb // P, H], F32)
        gamma_b = consts.tile([P, H], F32)
        beta_b = consts.tile([P, H], F32)

        # attention intermediates
        qT_sb = consts.tile([P, B, s], BF16)   # Q^T [d_local, b, s]
        kT_sb = consts.tile([P, B, s], BF16)   # K^T [d_local, b, s]
        # ones-augmented V (natural layout), per head: [p(s-inner), b, kb, 65]
        v_e = consts.tile([P, B, nkb, VPAD], FP8)
        v_o = consts.tile([P, B, nkb, VPAD], FP8)
        nc.vector.memset(v_e, 1.0)
        nc.vector.memset(v_o, 1.0)

        ctxf = consts.tile([P, B, ho, rpb], FP8)

        # PSUM: qk 2 banks (QKV drains + Wo + eb), s 2x2 banks, ctx 2 banks
        ps_qk = stack.enter_context(tc.tile_pool(name="ps_qk", bufs=2, space="PSUM"))
        ps_s = stack.enter_context(tc.tile_pool(name="ps_s", bufs=2, space="PSUM"))
        ps_ctx = stack.enter_context(tc.tile_pool(name="ps_ctx", bufs=2, space="PSUM"))
        ptile = stack.enter_context(tc.tile_pool(name="ptile", bufs=4))
        misc = stack.enter_context(tc.tile_pool(name="misc", bufs=3))
        fin = stack.enter_context(tc.tile_pool(name="fin", bufs=2))

        def qkv_proj(b, sc, w_sb, bias_sb, dst):
            sl = slice(sc * 512, (sc + 1) * 512)
            ps = ps_qk.tile([P, 512], F32, tag="qk")
            for o in range(ho):
                nc.tensor.matmul(
                    ps, lhsT=w_sb[:, o, :], rhs=xT_sb[:, b, o, sl],
                    start=(o == 0), stop=(o == ho - 1))
            nc.vector.tensor_tensor(
                dst[:, b, sl], ps,
                bias_sb[:, 0:1].to_broadcast((P, 512)),
                ALU.add)

        def qkv_vblock(b, kb):
            ksl = slice(kb * P, (kb + 1) * P)
            ps = ps_qk.tile([P, 512], F32, tag="qk")
            for o in range(ho):
                nc.tensor.matmul(
                    ps[:, 0:P], lhsT=xT_sb[:, b, o, ksl],
                    rhs=wv_sb[:, o, :],
                    start=(o == 0), stop=(o == ho - 1))
            nc.vector.tensor_tensor(
                v_e[:, b, kb, 0:64], ps[:, 0:64], bv_b[:, 0:64],
                ALU.add)
            nc.vector.tensor_tensor(
                v_o[:, b, kb, 0:64], ps[:, 64:128], bv_b[:, 64:128],
                ALU.add)

        def qkv_chunks(b):
            """QKV work split into ~1.7us PE chunks for interleaved emission."""
            kb_per_sc = nkb // (s // 512)
            for sc in range(s // 512):
                yield lambda sc=sc: qkv_proj(b, sc, wq_sb, bq_sb, qT_sb)
                yield lambda sc=sc: qkv_proj(b, sc, wk_sb, bk_sb, kT_sb)
                for kb in range(sc * kb_per_sc, (sc + 1) * kb_per_sc):
                    yield lambda kb=kb: qkv_vblock(b, kb)

        def qkv_stage(b):
            for ch in qkv_chunks(b):
                ch()

        def attn_qc(b, qc, a2a_in, filler=None):
            qsl = slice(qc * qw, (qc + 1) * qw)
            ctx_e = ps_ctx.tile([P, qw], F32, tag="ctx")
            ctx_o = ps_ctx.tile([P, qw], F32, tag="ctx")
            npair = nkb // 2
            for pr in range(npair):
                kb0, kb1 = 2 * pr, 2 * pr + 1
                for f in (filler or ()):
                    f(qc * npair * 2 + 2 * pr)
                sl0 = slice(kb0 * P, (kb0 + 1) * P)
                sl1 = slice(kb1 * P, (kb1 + 1) * P)
                # scores for both key blocks of the pair, per head; the two
                # heads auto-pack into row groups (0,0)/(64,0) of the array
                sp_e = ps_s.tile([P, 2, qw], F32, tag="s")
                sp_o = ps_s.tile([P, 2, qw], F32, tag="s")
                nc.tensor.matmul(
                    sp_e[:, 0, :], lhsT=kT_sb[0:64, b, sl0],
                    rhs=qT_sb[0:64, b, qsl], start=True, stop=True)
                nc.tensor.matmul(
                    sp_o[:, 0, :], lhsT=kT_sb[64:128, b, sl0],
                    rhs=qT_sb[64:128, b, qsl], start=True, stop=True)
                nc.tensor.matmul(
                    sp_e[:, 1, :], lhsT=kT_sb[0:64, b, sl1],
                    rhs=qT_sb[0:64, b, qsl], start=True, stop=True)
                nc.tensor.matmul(
                    sp_o[:, 1, :], lhsT=kT_sb[64:128, b, sl1],
                    rhs=qT_sb[64:128, b, qsl], start=True, stop=True)
                # exp of the raw logits: unnormalized exp(s/8) is O(1) and
                # sits comfortably in fp8e4m3's normal range
                pp_e = ptile.tile([P, 2, qw], FP8, tag="p")
                pp_o = ptile.tile([P, 2, qw], FP8, tag="p")
                if zero_mask:
                    nc.scalar.activation(pp_e, sp_e, AF.Exp,
                                         bias=zero_sb[:, 0:1], scale=SCALE)
                    nc.scalar.activation(pp_o, sp_o, AF.Exp,
                                         bias=zero_sb[:, 0:1], scale=SCALE)
                else:
                    for t, kb in ((0, kb0), (1, kb1)):
                        nc.scalar.activation(
                            pp_e[:, t, :], sp_e[:, t, :], AF.Exp,
                            bias=mask_sb[:, b, kb:kb + 1], scale=SCALE)
                        nc.scalar.activation(
                            pp_o[:, t, :], sp_o[:, t, :], AF.Exp,
                            bias=mask_sb[:, b, kb:kb + 1], scale=SCALE)
                # fp8 DoubleRow: both key blocks in one K=256 matmul
                nc.tensor.matmul(
                    ctx_e[0:65, :], lhsT=v_e[:, b, kb0:kb0 + 2, 0:65],
                    rhs=pp_e, start=(pr == 0), stop=(pr == npair - 1),
                    perf_mode=mybir.MatmulPerfMode.DoubleRow,
                    skip_group_check=True)
                nc.tensor.matmul(
                    ctx_o[0:65, :], lhsT=v_o[:, b, kb0:kb0 + 2, 0:65],
                    rhs=pp_o, start=(pr == 0), stop=(pr == npair - 1),
                    perf_mode=mybir.MatmulPerfMode.DoubleRow,
                    skip_group_check=True)
            # normalize: ctx[d, q] / denom[q]  (denom = row 64), emit to
            # the two destination shards this q-chunk covers
            for h, ctx_ps in enumerate((ctx_e, ctx_o)):
                rinv = misc.tile([1, qw], F32, tag="rinv")
                nc.vector.reciprocal(rinv, ctx_ps[64:65, :])
                rb = misc.tile([1, qw], BF16, tag="rb")
                nc.vector.tensor_scalar(rb, rinv, 256.0, None, ALU.mult)
                eb = ps_qk.tile([64, qw], F32, tag="qk")
                nc.tensor.matmul(
                    eb, lhsT=ones_sb[0:1, 0:64], rhs=rb,
                    start=True, stop=True)
                ctx_bf = misc.tile([64, qw], BF16, tag="cb")
                nc.vector.tensor_copy(out=ctx_bf, in_=ctx_ps[0:64, :])
                cn = misc.tile([64, qw], FP8, tag="cn")
                nc.vector.tensor_tensor(cn, ctx_bf, eb, ALU.mult)
                for half in range(qw // rpb):
                    dest = qc * (qw // rpb) + half
                    r0 = dest * P + h * 64
                    nc.sync.dma_start(
                        a2a_in[r0:r0 + 64, :],
                        cn[:, half * rpb:(half + 1) * rpb])

        def fin_wo(b, qt):
            """Wo projection + residual for one 128-row tile -> res tile."""
            res = fin.tile([P, H], F32, tag="res")
            for nch in range(H // 512):
                nsl = slice(nch * 512, (nch + 1) * 512)
                ps = ps_qk.tile([P, 512], F32, tag="qk")
                for t in range(ho // 2):
                    nc.tensor.matmul(
                        ps, lhsT=ctxf[:, b, 2 * t:2 * t + 2, qt * P:(qt + 1) * P],
                        rhs=wo_sb[:, 2 * t:2 * t + 2, nsl],
                        start=(t == 0), stop=(t == ho // 2 - 1),
                        perf_mode=mybir.MatmulPerfMode.DoubleRow)
                nc.vector.tensor_tensor(
                    res[:, nsl], ps, xres_sb[:, b, qt, nsl],
                    ALU.add)
            return res

        def fin_ln(b, qt, res):
            """LayerNorm over H (free axis) + store. DVE/Pool only -- the
            rstd is a Newton rsqrt so the ACT Exp table is never evicted."""
            stats = fin.tile([P, H // 512, 6], F32, tag="st")
            for g in range(H // 512):
                nc.vector.bn_stats(
                    stats[:, g, :], res[:, g * 512:(g + 1) * 512])
            mv = fin.tile([P, 2], F32, tag="mv")
            nc.vector.bn_aggr(out=mv, in_=stats)
            # y = rsqrt(var + eps) via Newton iterations (seed 1/65536:
            # rows are pre-scaled by 65536 and have ~unit variance)
            x = fin.tile([P, 1], F32, tag="x")
            nc.vector.tensor_tensor(x, mv[:, 1:2], eps_sb, ALU.add)
            y = fin.tile([P, 1], F32, tag="y")
            nc.vector.memset(y, 1.0 / 65536.0)
            t = fin.tile([P, 1], F32, tag="t")
            for _ in range(4):
                nc.vector.tensor_tensor(t, x, y, ALU.mult)
                nc.vector.tensor_tensor(t, t, y, ALU.mult)
                nc.vector.tensor_scalar(t, t, -0.5, 1.5, ALU.mult, ALU.add)
                nc.vector.tensor_tensor(y, y, t, ALU.mult)
            # (res - mu) * rstd in a single DVE pass
            nc.vector.tensor_scalar(
                res, res, mv[:, 0:1], y[:, 0:1],
                ALU.subtract, ALU.mult)
            outt = fin.tile([P, H], F32, tag="outt")
            nc.vector.tensor_tensor(outt, res, gamma_b, ALU.mult)
            nc.gpsimd.tensor_tensor(outt, outt, beta_b, ALU.add)
            nc.sync.dma_start(
                out[(b * (rpb // P) + qt) * P:(b * (rpb // P) + qt + 1) * P, :],
                outt)

        def do_a2a(pair):
            nc.gpsimd.collective_compute(
                "AllToAll", ALU.bypass,
                replica_groups=[list(range(NCORES))],
                ins=[pair[0][:].opt()], outs=[pair[1][:].opt()])

        def ctxf_load(b, a2a_out):
            for o in range(ho):
                nc.sync.dma_start(
                    ctxf[:, b, o, :],
                    a2a_out.rearrange("(o p) q -> p o q", p=P)[:, o, :])

        def make_filler(chunks, total_slots, start=0):
            """Spread chunk emission across attention kb slots >= start."""
            chunks = list(chunks)
            state = {"done": 0}
            n = len(chunks)

            def fill(g):
                if g < start:
                    return
                want = min(n, ((g - start + 1) * n) // max(1, total_slots - start))
                while state["done"] < want:
                    chunks[state["done"]]()
                    state["done"] += 1

            def flush():
                while state["done"] < len(chunks):
                    chunks[state["done"]]()
                    state["done"] += 1

            return fill, flush

        # preload the Exp activation table during the startup DMA wait
        warm = misc.tile([1, 1], F32, tag="warm")
        nc.scalar.activation(warm, eps_sb[0:1, :], AF.Exp)

        def fin_chunks(b):
            """The batch-b tail as filler chunks for the following phase."""
            resh = {}
            return [
                lambda: resh.__setitem__(0, fin_wo(b, 0)),
                lambda: fin_ln(b, 0, resh[0]),
                lambda: resh.__setitem__(1, fin_wo(b, 1)),
                lambda: fin_ln(b, 1, resh[1]),
            ]

        # software pipeline over 2*repeat phases: phase (it, b) runs batch-b
        # attention with (a) the next phase's QKV and (b) the previous
        # phase's Wo+LayerNorm tail interleaved into the kb loop, so the PE
        # stream never parks on a collective.
        nslots = qc_per_b * nkb
        qkv_stage(0)
        pending_fin = None
        for it in range(repeat):
            for b in range(B):
                a2a_i = dram.tile([NCORES * P, rpb], FP8, tag="a2ai",
                                  name=f"a2a_in_{it}_{b}")
                a2a_o = dram.tile([NCORES * P, rpb], FP8, tag="a2ao",
                                  name=f"a2a_out_{it}_{b}")
                last_phase = (it == repeat - 1) and (b == B - 1)
                fillers = []
                if not last_phase:
                    nb = (b + 1) % B
                    f_qkv, fl_qkv = make_filler(qkv_chunks(nb), nslots)
                    fillers.append(f_qkv)
                else:
                    fl_qkv = None
                if pending_fin is not None:
                    f_fin, fl_fin = make_filler(
                        pending_fin, nslots, start=int(nslots * 0.45))
                    fillers.append(f_fin)
                else:
                    fl_fin = None
                for qc in range(qc_per_b):
                    attn_qc(b, qc, a2a_i, filler=fillers)
                if fl_qkv:
                    fl_qkv()
                if fl_fin:
                    fl_fin()
                do_a2a((a2a_i, a2a_o))
                ctxf_load(b, a2a_o)
                if it == 0 and b == 0:
                    # tail-stage inputs (late emission => low DMA priority)
                    nc.sync.dma_start(
                        wo_sb, wo.rearrange("(o p) n -> p o n", p=P))
                    nc.sync.dma_start(
                        xres_sb,
                        xres.rearrange("(b r p) h -> p b r h", p=P, b=B))
                    nc.sync.dma_start(gamma_b, gamma[:, :])
                    nc.sync.dma_start(beta_b, beta[:, :])
                pending_fin = fin_chunks(b)
        for ch in pending_fin:
            ch()


def get_program(s=S, repeat=1, zero_mask=True):
    key = ("nc", s, repeat, zero_mask)
    if key not in _CACHE:
        _CACHE[key] = _build_program(s, repeat, zero_mask)
    return _CACHE[key]


def make_in_maps(hidden_states, attention_mask, Wq, bq, Wk, bk, Wv, bv, Wo, bo,
                 ln_gamma, ln_beta):
    """Host-side sharding: build the 8 per-core input maps."""
    bf = ml_dtypes.bfloat16
    hs = np.asarray(hidden_states, dtype=np.float32)
    b_, s_, h_ = hs.shape
    nkb = s_ // P
    rows = (b_ * s_) // NCORES
    rpb = rows // b_

    xT = np.ascontiguousarray(hs.transpose(0, 2, 1)).astype(bf)  # [B, H, S]
    Wq = np.asarray(Wq, np.float32)
    Wk = np.asarray(Wk, np.float32)
    Wv = np.asarray(Wv, np.float32)
    wo_f8 = np.ascontiguousarray(
        np.asarray(Wo, np.float32) * 256.0).astype(ml_dtypes.float8_e4m3)
    bq = np.asarray(bq, np.float32)
    bk = np.asarray(bk, np.float32)
    bv = np.asarray(bv, np.float32)
    bo = np.asarray(bo, np.float32)
    gamma_bc = np.ascontiguousarray(
        np.broadcast_to(np.asarray(ln_gamma, np.float32)[None, :], (P, H)))
    beta_bc = np.ascontiguousarray(
        np.broadcast_to(np.asarray(ln_beta, np.float32)[None, :], (P, H)))
    mask = np.asarray(attention_mask, np.float32).reshape(b_, s_)
    maskT = np.ascontiguousarray(
        mask.reshape(b_, nkb, P).transpose(0, 2, 1))  # [B, P, nkb]

    in_maps = []
    for c in range(NCORES):
        d0 = c * P
        rsl = slice(c * rpb, (c + 1) * rpb)
        xres_c = np.concatenate([hs[b, rsl, :] for b in range(b_)], axis=0)
        in_maps.append({
            "xT": xT,
            "wq": np.ascontiguousarray(Wq[:, d0:d0 + P]).astype(bf),
            "wk": np.ascontiguousarray(Wk[:, d0:d0 + P]).astype(bf),
            "wv": np.ascontiguousarray(Wv[:, d0:d0 + P]).astype(bf),
            "wo": wo_f8,
            "bq": np.ascontiguousarray(bq[d0:d0 + P]),
            "bk": np.ascontiguousarray(bk[d0:d0 + P]),
            "bv": np.ascontiguousarray(
                np.broadcast_to(bv[d0:d0 + P][None, :], (P, P))),
            "maskT": maskT,
            "xres": np.ascontiguousarray(
                (xres_c + bo[None, :]) * 65536.0),
            "gamma": gamma_bc,
            "beta": beta_bc,
        })
    return in_maps


def assemble_output(results, b_=B, s_=S, h_=H):
    rows = (b_ * s_) // NCORES
    rpb = rows // b_
    out = np.empty((b_, s_, h_), np.float32)
    for c in range(NCORES):
        r = np.asarray(results[c]["out"], np.float32)
        for b in range(b_):
            out[b, c * rpb:(c + 1) * rpb, :] = r[b * rpb:(b + 1) * rpb, :]
    return out


def kernel(**inputs):
    zero_mask = bool(np.all(np.asarray(inputs["attention_mask"]) == 0.0))
    nc = get_program(S, zero_mask=zero_mask)
    in_maps = make_in_maps(**inputs)
    res = run_bass_kernel_spmd(nc, in_maps, list(range(NCORES)))
    return assemble_output(res.results)
